# revision 16
# baseline (speedup 1.0000x reference)
"""Trainium2 Bass kernel for GQA attention (B=2, T=4096, D=2048, N=8 q-heads,
K=1 kv-head, H=256) with RoPE + causal mask + output projection.

Sharding: data-parallel on batch (2) x tensor-parallel on query heads
(4 groups of 2 heads) = 8 cores. Each core computes a partial output
y_c = sum_{n in its 2 heads} softmax(q_n k^T) v @ out_w[n] for its batch;
the host sums the 4 partials per batch.

K/V projection dedup: the 4 cores of a batch would otherwise each compute
the identical K/V projection. Instead the D=2048 contraction is split in
4: the host permutes the 16 D-chunks per core so the core's own quarter
is always chunks 0..3 (the same permutation is applied to x^T, q_w and
kv_w, and a sum over D is order-invariant, so everything else is
unchanged and the NEFF stays SPMD-uniform). Each core computes K/V
partial sums for a tile over its 4 chunks only, and an AllReduce(add)
over its 4-core replica group completes the projection. The collective
is launched two tiles ahead so its latency hides under attention
compute; tile 0 is computed fully locally to avoid a startup bubble.

The device kernel is identical on every core (single NEFF, SPMD); per-core
behaviour comes only from per-core input data:
  xt   [2048, 4096] bf16 : x[b]^T  (pre-transposed + bf16, D-chunks permuted)
  qw   [2, 2048, 256] bf16 : q_w for the core's 2 heads, pre-scaled by H^-0.5
  kvw  [2, 2048, 256] bf16 : k/v projection weights (shared kv head)
  outw [2, 256, 2048] bf16 : out_w for the core's 2 heads
  cost/sint [128, 4096] f32 : RoPE cos/sin tables (timescale j x position t)
Output: y [4096, 2048] bf16 partial (summed in f32 on host).

Flash-attention layout: everything transposed (S^T = K^T^T-contraction) so
softmax statistics land in matmuls:
  K^T,Q^T [h, t] from projections directly; logits S^T [s-chunk 128, t 512]
  in PSUM; exp on ACT -> P^T bf16; PV as pt-stationary matmul giving
  O [t-sub, h | denom] accumulated over s-chunks in PSUM; denominator via
  a constant-1 column appended to V; normalization by per-partition DVE
  scale, then PE transpose to O^T for the output projection.

Scheduling: the (head, chunk) loop is software-pipelined one step deep --
QK(k+1) is issued on the PE before PV(k) -- so the QK->exp->PV chain
latency (ACT engine) is hidden behind the next chunk's QK matmuls.
Normalization of query sub-block ts is issued 2 steps after the chunk that
finalizes its PSUM row, and the output projection streams out per 128-row
sub-block as soon as both heads' normalized O^T slices exist.
"""

import os
from collections import deque

import numpy as np
import ml_dtypes

B, T, D, N, H = 2, 4096, 2048, 8, 256
NCORES = 8
HH = H // 2  # 128, also the RoPE pair offset and partition size
TQ = 512     # query-tile columns (moving dim of logits matmul)
NT = T // TQ # 8 query tiles
NDC = D // 128  # 16 contraction chunks over D
NDQ = NDC // 4  # 4 chunks per core's D-quarter

_CACHE = {}
LAST_RESULT = None  # BassKernelResults of the most recent device run (for test harness)


def _split_excess_waits(nc):
    """The walrus in this container accepts at most 1 sync-wait per
    instruction (2 for EventSemaphore); Tile attaches one wait per producer
    semaphore. Hoist excess waits onto injected same-engine NOPs immediately
    before the instruction (engine queues are in-order, so waiting A then B
    sequentially == waiting {A,B} at once)."""
    import bass_rust
    import concourse.mybir as mybir

    n_split = 0
    for f in nc.m.functions:
        for bb in f.blocks:
            insts = bb.instructions
            out = []
            changed = False
            for inst in insts:
                si = inst.sync_info
                waits = list(si.on_wait) if si is not None and si.on_wait else []
                cap = 2 if isinstance(inst, mybir.InstEventSemaphore) else 1
                if len(waits) > cap:
                    changed = True
                    for w in waits[:-cap]:
                        nop = mybir.InstNoOp(
                            name=f"waitsplit_{n_split}", ins=[], outs=[]
                        )
                        n_split += 1
                        nop.engine = inst.engine
                        nop.sync_info = bass_rust.SyncInfo(on_wait=[w], on_update=[])
                        out.append(nop)
                    inst.sync_info = bass_rust.SyncInfo(
                        on_wait=waits[-cap:], on_update=si.on_update
                    )
                out.append(inst)
            if changed:
                insts[:] = out
                if bb.instructions[0].name != out[0].name or len(bb.instructions) != len(out):
                    raise RuntimeError("basic block instruction list not live-mutable")
    return n_split


def _rope_core(nc, tmps, c0, c1, cos_s, sin_s, out0, out1, L):
    """out0 = c0*cos - c1*sin ; out1 = c1*cos + c0*sin on the DVE.
    c0/c1: [128, L] f32 SBUF."""
    import concourse.mybir as mybir

    t0 = tmps.tile([128, TQ], mybir.dt.float32, tag="ropetmp", bufs=2)
    t1 = tmps.tile([128, TQ], mybir.dt.float32, tag="ropetmp", bufs=2)
    nc.vector.tensor_mul(t0[:, :L], c0, cos_s)
    nc.vector.tensor_mul(t1[:, :L], c1, sin_s)
    nc.vector.tensor_sub(out0, t0[:, :L], t1[:, :L])
    t2 = tmps.tile([128, TQ], mybir.dt.float32, tag="ropetmp", bufs=2)
    t3 = tmps.tile([128, TQ], mybir.dt.float32, tag="ropetmp", bufs=2)
    nc.vector.tensor_mul(t2[:, :L], c1, cos_s)
    nc.vector.tensor_mul(t3[:, :L], c0, sin_s)
    nc.vector.tensor_add(out1, t2[:, :L], t3[:, :L])


def _rope_pair(nc, tmps, p0, p1, cos_s, sin_s, out0, out1):
    """RoPE half-pair from PSUM: p0/p1 are [128, L] f32 PSUM; copied to
    SBUF first (frees the PSUM bank after ~1 op instead of after 4 DVE
    ops). cos/sin: [128, L] f32 SBUF, out0/out1: [128, L] bf16 SBUF."""
    import concourse.mybir as mybir

    L = p0.shape[-1]
    c0 = tmps.tile([128, TQ], mybir.dt.float32, tag="projc")
    c1 = tmps.tile([128, TQ], mybir.dt.float32, tag="projc")
    nc.scalar.copy(c0[:, :L], p0)
    nc.vector.tensor_copy(c1[:, :L], p1)
    _rope_core(nc, tmps, c0[:, :L], c1[:, :L], cos_s, sin_s, out0, out1, L)


def _build_nc():
    import concourse.bass as bass
    import concourse.mybir as mybir
    import concourse.tile as tile

    F32 = mybir.dt.float32
    BF16 = mybir.dt.bfloat16
    EXP = mybir.ActivationFunctionType.Exp
    RG = [[0, 1, 2, 3], [4, 5, 6, 7]]

    nc = bass.Bass(num_devices=NCORES)
    # All inputs are pre-arranged on the host to the exact SBUF layout so
    # every DMA moves >=2KB-contiguous per-partition lines at full rate.
    xt = nc.dram_tensor("xt", [NT, 128, NDC, TQ], BF16, kind="ExternalInput")
    qw = nc.dram_tensor("qw", [128, 2, NDC, H], BF16, kind="ExternalInput")
    kvw = nc.dram_tensor("kvw", [128, 2, NDC, H], BF16, kind="ExternalInput")
    outw = nc.dram_tensor("outw", [128, 2, 2, D], BF16, kind="ExternalInput")
    cost = nc.dram_tensor("cost", [HH, T], F32, kind="ExternalInput")
    sint = nc.dram_tensor("sint", [HH, T], F32, kind="ExternalInput")
    y = nc.dram_tensor("y", [T, D], BF16, kind="ExternalOutput")

    with tile.TileContext(nc) as tc:
        with (
            tc.tile_pool(name="const", bufs=1) as constp,
            tc.tile_pool(name="persist", bufs=1) as persist,
            tc.tile_pool(name="stream", bufs=3) as stream,
            tc.tile_pool(name="cstream", bufs=2) as cstream,
            tc.tile_pool(name="qtp", bufs=4) as qtp,
            tc.tile_pool(name="tmps", bufs=3) as tmps,
            tc.tile_pool(name="ptp", bufs=3) as ptp,
            tc.tile_pool(name="otp", bufs=2) as otp,
            tc.tile_pool(name="ysp", bufs=2) as ysp,
            tc.tile_pool(name="sndp", bufs=1) as sndp,
            tc.tile_pool(name="kvfp", bufs=1) as kvfp,
            tc.tile_pool(name="dram", bufs=2, space="DRAM") as dram,
            tc.tile_pool(name="psum", bufs=1, space="PSUM") as psum,
        ):
            # --- constants -------------------------------------------------
            # Causal triangle (only the 128-wide diagonal block ever needs
            # masking): cmask[p, c] = 0 if c >= p else -1e30.
            cmask = constp.tile([128, 128], F32)
            nc.gpsimd.memset(cmask, 0.0)
            nc.gpsimd.affine_select(
                out=cmask,
                in_=cmask,
                compare_op=mybir.AluOpType.is_ge,
                fill=-1.0e30,
                base=0,
                pattern=[[1, 128]],
                channel_multiplier=-1,
            )
            ident = constp.tile([128, 128], BF16)
            from concourse.masks import make_identity

            make_identity(nc, ident)

            # HAM warm-up: the PE clock-gate defaults to half rate and takes
            # ~3.4us of sustained matmul activity to open. The first real
            # matmul waits a few us for the kvs/xt DMAs, so burn that window
            # on throwaway matmuls to enter the loop at full clock.
            warm = psum.tile([128, 128], F32, tag="projqk", bufs=2, name="warm")
            for _ in range(40):
                nc.tensor.matmul(warm, lhsT=ident, rhs=ident, start=True, stop=True)

            # --- resident weights / tables --------------------------------
            from concourse.tile import add_dep_helper

            # Startup DMA choreography: everything issued at t=0 shares HBM
            # bandwidth, so the critical first-tile inputs (K weights, first
            # x^T slice) go first; the bulk weights are chained behind them
            # in the order the PE consumes them (Q weights ~7us in, V
            # weights ~20us, x^T tile 1 ~20us, out weights ~35us).
            kvs = constp.tile([128, 2, NDC, H], BF16)
            d_crit = []
            for dg in range(4):
                dd = nc.sync.dma_start(
                    out=kvs[:, 0, 4 * dg : 4 * (dg + 1)],
                    in_=kvw[:, 0, 4 * dg : 4 * (dg + 1)],
                )
                d_crit.append(dd)
            xts_t = [None] * NT
            xts_t[0] = stream.tile([128, NDC, TQ], BF16, tag="xts", name="xts")
            d_x0 = []
            for dg in range(4):
                dd = nc.sync.dma_start(
                    out=xts_t[0][:, 4 * dg : 4 * (dg + 1), :],
                    in_=xt[0, :, 4 * dg : 4 * (dg + 1), :],
                )
                d_x0.append(dd)
            css = [None] * NT
            sns = [None] * NT

            def cs_dma(j, deps=()):
                css[j] = cstream.tile([128, TQ], F32, tag="cs", name="cs")
                sns[j] = cstream.tile([128, TQ], F32, tag="sn", name="sn")
                jsl = slice(j * TQ, (j + 1) * TQ)
                d1 = nc.sync.dma_start(out=css[j], in_=cost[:, jsl])
                d2 = nc.sync.dma_start(out=sns[j], in_=sint[:, jsl])
                for dep in deps:
                    add_dep_helper(d1.ins, dep.ins, reason="defer cos/sin")
                    add_dep_helper(d2.ins, dep.ins, reason="defer cos/sin")

            cs_dma(0, deps=(d_x0[-1],))
            qws = constp.tile([128, 2, NDC, H], BF16)
            d_qw = []
            for n in (0, 1):
                for hg in (0, 1):
                    dd = nc.sync.dma_start(
                        out=qws[:, n, 8 * hg : 8 * (hg + 1)],
                        in_=qw[:, n, 8 * hg : 8 * (hg + 1)],
                    )
                    add_dep_helper(dd.ins, d_crit[-1].ins, reason="after K weights")
                    add_dep_helper(dd.ins, d_x0[-1].ins, reason="after first x")
                    d_qw.append(dd)
            d_kv1 = []
            for hg in (0, 1):
                dd = nc.sync.dma_start(
                    out=kvs[:, 1, 8 * hg : 8 * (hg + 1)],
                    in_=kvw[:, 1, 8 * hg : 8 * (hg + 1)],
                )
                add_dep_helper(dd.ins, d_qw[-1].ins, reason="after Q weights")
                d_kv1.append(dd)
            # x^T tile 1 (needed ~20us in for the tile-1 K/V partials)
            d_x1 = []
            xts_t[1] = stream.tile([128, NDC, TQ], BF16, tag="xts", name="xts")
            for dg in range(4):
                dd = nc.sync.dma_start(
                    out=xts_t[1][:, 4 * dg : 4 * (dg + 1), :],
                    in_=xt[1, :, 4 * dg : 4 * (dg + 1), :],
                )
                add_dep_helper(dd.ins, d_qw[-1].ins, reason="after Q weights")
                d_x1.append(dd)
            cs_dma(1, deps=(d_x1[-1],))
            ows = constp.tile([128, 2, 2, D], BF16)
            for hg in (0, 1):
                dd = nc.sync.dma_start(out=ows[:, hg], in_=outw[:, hg])
                add_dep_helper(dd.ins, d_kv1[-1].ins, reason="after V weights")

            # K^T halves [h-half, t] and V chunks [s-in-chunk, h | ones],
            # grown per tile. The 257th column of each V chunk is constant 1.0
            # so the PV matmul accumulates the softmax denominator for free.
            VN = H + 1  # 257
            kts = persist.tile([128, 2, T], BF16)
            vs = persist.tile([128, T // 128, VN], BF16)
            nc.vector.memset(vs[:, :, H : H + 1], 1.0)

            # Per-tile K/V via collective: snd packs [K0|K1|V0..V3] f32.
            kv_out = [None] * NT  # gathered DRAM tiles per tile j

            def kv_partial_cc(j):
                """K/V partial projection for tile j over this core's 4
                D-chunks, packed and AllReduce'd across the replica group."""
                ctx = nc.named_scope(f"t{j}_kvcc"); ctx.__enter__()
                snd = sndp.tile([128, 2048], BF16, tag="snd", name="snd")
                kp0 = psum.tile([128, TQ], F32, tag="projqk", bufs=2)
                kp1 = psum.tile([128, TQ], F32, tag="projqk", bufs=2)
                for hh, kp in ((0, kp0), (1, kp1)):
                    for d in range(NDQ):
                        nc.tensor.matmul(
                            kp,
                            lhsT=kvs[:, 0, d, hh * 128 : (hh + 1) * 128],
                            rhs=xts_t[j][:, d, :],
                            start=(d == 0),
                            stop=(d == NDQ - 1),
                        )
                nc.vector.tensor_copy(snd[:, 0:512], kp0)
                nc.vector.tensor_copy(snd[:, 512:1024], kp1)
                for g in range(4):
                    vp = psum.tile([128, H], F32, tag="projqk", bufs=2)
                    for d in range(NDQ):
                        nc.tensor.matmul(
                            vp,
                            lhsT=xts_t[j][:, d, g * 128 : (g + 1) * 128],
                            rhs=kvs[:, 1, d, :],
                            start=(d == 0),
                            stop=(d == NDQ - 1),
                        )
                    nc.vector.tensor_copy(
                        snd[:, 1024 + 256 * g : 1280 + 256 * g], vp
                    )
                cc_in = dram.tile([128, 2048], BF16, tag="ccin", name="cc_in")
                cc_out = dram.tile([128, 2048], BF16, tag="ccout", name="cc_out")
                nc.sync.dma_start(out=cc_in[:], in_=snd[:])
                nc.gpsimd.collective_compute(
                    "AllReduce",
                    mybir.AluOpType.add,
                    replica_groups=RG,
                    ins=[cc_in.opt()],
                    outs=[cc_out.opt()],
                )
                kv_out[j] = cc_out
                ctx.__exit__(None, None, None)

            for i in range(NT):
                tsl = slice(i * TQ, (i + 1) * TQ)
                cos_sl = css[i]
                sin_sl = sns[i]
                # prefetches: x^T two tiles ahead, cos/sin one ahead, and
                # the gathered K/V of this tile back from DRAM.
                if i + 2 < NT:
                    xts_t[i + 2] = stream.tile(
                        [128, NDC, TQ], BF16, tag="xts", name="xts"
                    )
                    for dg in range(4):
                        nc.sync.dma_start(
                            out=xts_t[i + 2][:, 4 * dg : 4 * (dg + 1), :],
                            in_=xt[i + 2, :, 4 * dg : 4 * (dg + 1), :],
                        )
                if i + 1 < NT and i > 0:
                    cs_dma(i + 1)
                kvf = None
                if i > 0:
                    kvf = kvfp.tile([128, 2048], BF16, tag="kvf", name="kvf")
                    nc.sync.dma_start(out=kvf[:], in_=kv_out[i][:])

                ctx_proj = nc.named_scope(f"t{i}_proj"); ctx_proj.__enter__()
                xts = xts_t[i]
                if i == 0:
                    # ---- tile 0: full local K^T projection + RoPE --------
                    kp0 = psum.tile([128, TQ], F32, tag="projqk", bufs=2)
                    kp1 = psum.tile([128, TQ], F32, tag="projqk", bufs=2)
                    for hh, kp in ((0, kp0), (1, kp1)):
                        for d in range(NDC):
                            nc.tensor.matmul(
                                kp,
                                lhsT=kvs[:, 0, d, hh * 128 : (hh + 1) * 128],
                                rhs=xts[:, d, :],
                                start=(d == 0),
                                stop=(d == NDC - 1),
                            )
                    _rope_pair(
                        nc, tmps, kp0, kp1, cos_sl, sin_sl,
                        kts[:, 0, tsl], kts[:, 1, tsl],
                    )
                else:
                    # ---- K^T RoPE from the gathered projection -----------
                    _rope_core(
                        nc, tmps, kvf[:, 0:512], kvf[:, 512:1024],
                        cos_sl, sin_sl,
                        kts[:, 0, tsl], kts[:, 1, tsl], TQ,
                    )
                    # launch the look-ahead K/V partial + collective early
                    # so the AllReduce has the whole tile to complete in
                    if i + 2 < NT:
                        kv_partial_cc(i + 2)

                # ---- Q^T projections + RoPE (2 heads) --------------------
                qt = []
                for n in (0, 1):
                    qp0 = psum.tile([128, TQ], F32, tag="projqk", bufs=2)
                    qp1 = psum.tile([128, TQ], F32, tag="projqk", bufs=2)
                    for hh, qp in ((0, qp0), (1, qp1)):
                        for d in range(NDC):
                            nc.tensor.matmul(
                                qp,
                                lhsT=qws[:, n, d, hh * 128 : (hh + 1) * 128],
                                rhs=xts[:, d, :],
                                start=(d == 0),
                                stop=(d == NDC - 1),
                            )
                    qtn = qtp.tile([128, 2, TQ], BF16, tag="qt")
                    _rope_pair(
                        nc, tmps, qp0, qp1, cos_sl, sin_sl,
                        qtn[:, 0, :], qtn[:, 1, :],
                    )
                    qt.append(qtn)

                # ---- K/V partials + collective (tile 0: both pipelined
                # tiles launch here, after the weights have streamed in) ---
                if i == 0:
                    kv_partial_cc(1)
                    kv_partial_cc(2)

                # ---- V into the persistent chunk buffer ------------------
                if i == 0:
                    for ts in range(4):
                        vp = psum.tile([128, H], F32, tag="projqk", bufs=2)
                        for d in range(NDC):
                            nc.tensor.matmul(
                                vp,
                                lhsT=xts[:, d, ts * 128 : (ts + 1) * 128],
                                rhs=kvs[:, 1, d, :],
                                start=(d == 0),
                                stop=(d == NDC - 1),
                            )
                        nc.vector.tensor_copy(vs[:, 4 * i + ts, 0:H], vp)
                else:
                    for ts in range(4):
                        nc.vector.tensor_copy(
                            vs[:, 4 * i + ts, 0:H],
                            kvf[:, 1024 + 256 * ts : 1280 + 256 * ts],
                        )

                ctx_proj.__exit__(None, None, None)

                # ---- attention, software-pipelined -----------------------
                # O[t-sub, h|denom] accumulates per 128-row query sub-block in
                # PSUM over s-chunks: lhsT = P^T[s, t-sub], rhs = [V | 1].
                # The flat (head, chunk) step sequence runs QK(step j+1)
                # before PV(step j); normalization of sub-block ts is emitted
                # two steps after the chunk that finalizes it, and the output
                # projection for ts streams out right after head 1's ts.
                ctx_attn = nc.named_scope(f"t{i}_attn"); ctx_attn.__enter__()
                nchunks = 4 * i + 4
                ots = [
                    otp.tile([128, 2, TQ], BF16, tag=f"ot{n}", name=f"ot{n}")
                    for n in (0, 1)
                ]
                o_ps = {}
                norm_fifo = deque()
                state = {"pv": 0}

                def out_proj_ts(ts):
                    """Output projection for rows [i*TQ+ts*128, +128)."""
                    ys = ysp.tile([128, D], BF16, tag="ys", name="ys")
                    for dc in range(4):
                        py = psum.tile(
                            [128, 512], F32, tag=f"o{ts}", name="py"
                        )
                        mm = 0
                        for n in (0, 1):
                            for hh in (0, 1):
                                nc.tensor.matmul(
                                    py,
                                    lhsT=ots[n][:, hh, ts * 128 : (ts + 1) * 128],
                                    rhs=ows[:, n, hh, dc * 512 : (dc + 1) * 512],
                                    start=(mm == 0),
                                    stop=(mm == 3),
                                )
                                mm += 1
                        nc.vector.tensor_copy(ys[:, dc * 512 : (dc + 1) * 512], py)
                    nc.sync.dma_start(
                        out=y[i * TQ + ts * 128 : i * TQ + (ts + 1) * 128, :],
                        in_=ys,
                    )

                def emit_norm(n, ts):
                    """Normalize o_ps[n][ts] by its denominator column and
                    transpose into ots[n]; stream the output projection once
                    head 1's slice lands."""
                    rd = tmps.tile([128, 1], F32, tag="rd", bufs=4)
                    nc.vector.reciprocal(rd, o_ps[n][ts][:, H : H + 1])
                    ob = tmps.tile([128, H], BF16, tag="ob", bufs=3)
                    nc.vector.tensor_scalar_mul(ob, o_ps[n][ts][:, 0:H], rd)
                    for hh in (0, 1):
                        tp = psum.tile(
                            [128, 128], BF16, tag="projqk", bufs=2, name="tp"
                        )
                        nc.tensor.transpose(
                            tp, ob[:, 128 * hh : 128 * (hh + 1)], ident
                        )
                        nc.vector.tensor_copy(
                            ots[n][:, hh, 128 * ts : 128 * (ts + 1)], tp
                        )
                    if n == 1:
                        out_proj_ts(ts)

                def emit_pv(n, k, pt):
                    """PV matmuls for chunk k of head n, then any normalize
                    whose finalizing chunk's PV was emitted a step ago."""
                    if n not in o_ps:
                        o_ps[n] = [
                            psum.tile([128, VN], F32, tag=f"o{ts}", name=f"o{ts}")
                            for ts in range(4)
                        ]
                    q_ = max(0, k - 4 * i)
                    for ts in range(q_, 4):
                        nc.tensor.matmul(
                            o_ps[n][ts],
                            lhsT=pt[:, 128 * ts : 128 * (ts + 1)],
                            rhs=vs[:, k, :],
                            start=(k == 0),
                            stop=(k == 4 * i + ts),
                        )
                    state["pv"] += 1
                    if k >= 4 * i:
                        norm_fifo.append((n, k - 4 * i, state["pv"]))
                    while norm_fifo and norm_fifo[0][2] <= state["pv"] - 1:
                        nn, ts, _ = norm_fifo.popleft()
                        emit_norm(nn, ts)

                prev = None
                for n in (0, 1):
                    for k in range(nchunks):
                        q_ = max(0, k - 4 * i)
                        col0 = 128 * q_
                        ksl = slice(k * 128, (k + 1) * 128)
                        pl = psum.tile([128, TQ], F32, tag="work", bufs=2, name="pl")
                        nc.tensor.matmul(
                            pl[:, col0:],
                            lhsT=kts[:, 0, ksl],
                            rhs=qt[n][:, 0, col0:],
                            start=True,
                            stop=False,
                        )
                        nc.tensor.matmul(
                            pl[:, col0:],
                            lhsT=kts[:, 1, ksl],
                            rhs=qt[n][:, 1, col0:],
                            start=False,
                            stop=True,
                        )
                        if k >= 4 * i:
                            # only the 128-wide diagonal block needs masking
                            nc.vector.tensor_add(
                                pl[:, col0 : col0 + 128],
                                pl[:, col0 : col0 + 128],
                                cmask,
                            )
                        pt = ptp.tile([128, TQ], BF16, tag="pt", bufs=4, name="pt")
                        nc.scalar.activation(pt[:, col0:], pl[:, col0:], EXP)
                        if prev is not None:
                            emit_pv(*prev)
                        prev = (n, k, pt)
                emit_pv(*prev)
                while norm_fifo:
                    nn, ts, _ = norm_fifo.popleft()
                    emit_norm(nn, ts)
                ctx_attn.__exit__(None, None, None)
    n = _split_excess_waits(nc)
    print(f"kernel build: split {n} excess waits")
    return nc


def _is_causal(mask):
    """mask: [B, T, T] bool — check it's exactly the causal tril mask."""
    tri = np.tril(np.ones((T, T), dtype=bool))
    return all(np.array_equal(mask[b], tri) for b in range(mask.shape[0]))


def _numpy_reference(x, segment_pos, attn_mask, q_w, kv_w, out_w):
    """Slow exact fallback for non-causal masks (matches reference.py)."""
    x = np.asarray(x, np.float32)
    out = np.zeros((B, T, D), np.float32)
    j = np.arange(HH, dtype=np.float32)
    timescale = 10000.0 ** (2.0 * j / H)
    for b in range(B):
        ang = segment_pos[b][:, None].astype(np.float32) / timescale[None, :]
        cos, sin = np.cos(ang), np.sin(ang)  # [T, 128]
        k = x[b] @ kv_w[0, 0]  # [T, H]
        v = x[b] @ kv_w[1, 0]
        k = np.concatenate(
            [k[:, :HH] * cos - k[:, HH:] * sin, k[:, HH:] * cos + k[:, :HH] * sin], 1
        )
        for n in range(N):
            q = x[b] @ q_w[n]
            q = np.concatenate(
                [q[:, :HH] * cos - q[:, HH:] * sin, q[:, HH:] * cos + q[:, :HH] * sin],
                1,
            ) * (H ** -0.5)
            logits = q @ k.T  # [T, T]
            logits = np.where(attn_mask[b], logits, -2.3819763e38)
            logits -= logits.max(-1, keepdims=True)
            p = np.exp(logits)
            p /= p.sum(-1, keepdims=True)
            out[b] += (p.astype(np.float32) @ v) @ out_w[n]
    return out


def kernel(x, segment_pos, attn_mask, q_w, kv_w, out_w):
    global LAST_RESULT
    x = np.asarray(x)
    segment_pos = np.asarray(segment_pos)
    attn_mask = np.asarray(attn_mask)
    q_w = np.asarray(q_w)
    kv_w = np.asarray(kv_w)
    out_w = np.asarray(out_w)
    assert x.shape == (B, T, D) and q_w.shape == (N, D, H)

    if not _is_causal(attn_mask):
        return _numpy_reference(x, segment_pos, attn_mask, q_w, kv_w, out_w)

    from concourse.bass_utils import run_bass_kernel_spmd

    if "nc" not in _CACHE:
        _CACHE["nc"] = _build_nc()
    nc = _CACHE["nc"]

    bf16 = ml_dtypes.bfloat16

    # D-chunk permutation per core quarter: own 4 chunks first.
    perms = [
        [*range(4 * dq, 4 * dq + 4)] + [d for d in range(NDC) if d // 4 != dq]
        for dq in range(4)
    ]

    def dxh_pre(w, perm):  # [2, D, H] -> [128, 2, NDC(perm), H]
        return np.ascontiguousarray(
            w.reshape(2, NDC, 128, H).transpose(2, 0, 1, 3)[:, :, perm]
        ).astype(bf16)

    # Per-batch host prep
    xts, coss, sins = [], [], []
    j = np.arange(HH, dtype=np.float32)
    timescale = 10000.0 ** (2.0 * j / H)
    for b in range(B):
        # x[b] [T, D] -> x^T tiles [NT, 128(dp), NDC, TQ]
        xtp = np.ascontiguousarray(
            x[b].T.reshape(NDC, 128, NT, TQ).transpose(2, 1, 0, 3)
        )
        xts.append([
            np.ascontiguousarray(xtp[:, :, perm]).astype(bf16) for perm in perms
        ])
        ang = segment_pos[b][None, :].astype(np.float32) / timescale[:, None]
        coss.append(np.cos(ang).astype(np.float32))
        sins.append(np.sin(ang).astype(np.float32))
    kvw_all = [dxh_pre(kv_w[:, 0], perm) for perm in perms]
    qw_scaled = q_w * np.float32(H ** -0.5)  # [N, D, H]
    # out_w [n, H, D] -> [128(hp), 2(n), 2(hh), D]
    outw_all = [
        np.ascontiguousarray(
            out_w[2 * m : 2 * m + 2].reshape(2, 2, 128, D).transpose(2, 0, 1, 3)
        ).astype(bf16)
        for m in range(4)
    ]
    qw_all = [dxh_pre(qw_scaled[2 * m : 2 * m + 2], perms[m]) for m in range(4)]

    in_maps = []
    for c in range(NCORES):
        b, m = c // 4, c % 4
        in_maps.append(
            {
                "xt": xts[b][m],
                "qw": qw_all[m],
                "kvw": kvw_all[m],
                "outw": outw_all[m],
                "cost": coss[b],
                "sint": sins[b],
            }
        )

    trace = bool(int(os.environ.get("KERNEL_TRACE", "0")))
    res = run_bass_kernel_spmd(nc, in_maps, core_ids=list(range(NCORES)), trace=trace)
    LAST_RESULT = res

    out = np.zeros((B, T, D), np.float32)
    for c in range(NCORES):
        out[c // 4] += res.results[c]["y"].astype(np.float32)
    return out


# revision 20
# speedup vs baseline: 1.0194x; 1.0194x over previous
"""Trainium2 Bass kernel for GQA attention (B=2, T=4096, D=2048, N=8 q-heads,
K=1 kv-head, H=256) with RoPE + causal mask + output projection.

Sharding: data-parallel on batch (2) x tensor-parallel on query heads
(4 groups of 2 heads) = 8 cores. Each core computes a partial output
y_c = sum_{n in its 2 heads} softmax(q_n k^T) v @ out_w[n] for its batch;
the host sums the 4 partials per batch. (A cross-core AllReduce KV-dedup
was tried and reverted: collective SDMA traffic trips a GPIO power
throttle that caps the PE clock at 13/16 for most of the run, costing
more than the deduplicated projection work saved.)

The device kernel is identical on every core (single NEFF, SPMD); per-core
behaviour comes only from per-core input data:
  xt   [2048, 4096] bf16 : x[b]^T  (pre-transposed + bf16 on host)
  qw   [2, 2048, 256] bf16 : q_w for the core's 2 heads, pre-scaled by H^-0.5
  kvw  [2, 2048, 256] bf16 : k/v projection weights (shared kv head)
  outw [2, 256, 2048] bf16 : out_w for the core's 2 heads
  cost/sint [128, 4096] f32 : RoPE cos/sin tables (timescale j x position t)
Output: y [4096, 2048] bf16 partial (summed in f32 on host).

Flash-attention layout: everything transposed (S^T = K^T^T-contraction) so
softmax statistics land in matmuls:
  K^T,Q^T [h, t] from projections directly; logits S^T [s-chunk 128, t 512]
  in PSUM; exp on ACT -> P^T bf16; PV as pt-stationary matmul giving
  O [t-sub, h | denom] accumulated over s-chunks in PSUM; denominator via
  a constant-1 column appended to V; normalization by per-partition DVE
  scale, then PE transpose to O^T for the output projection.

Scheduling: the (head, chunk) loop is software-pipelined one step deep --
QK(k+1) is issued on the PE before PV(k) -- so the QK->exp->PV chain
latency (ACT engine) is hidden behind the next chunk's QK matmuls.
Normalization of query sub-block ts is issued 2 steps after the chunk that
finalizes its PSUM row, and the output projection streams out per 128-row
sub-block as soon as both heads' normalized O^T slices exist.
"""

import os
from collections import deque

import numpy as np
import ml_dtypes

B, T, D, N, H = 2, 4096, 2048, 8, 256
NCORES = 8
HH = H // 2  # 128, also the RoPE pair offset and partition size
TQ = 512     # query-tile columns (moving dim of logits matmul)
NT = T // TQ # 8 query tiles
NDC = D // 128  # 16 contraction chunks over D

_CACHE = {}
LAST_RESULT = None  # BassKernelResults of the most recent device run (for test harness)


def _split_excess_waits(nc):
    """The walrus in this container accepts at most 1 sync-wait per
    instruction (2 for EventSemaphore); Tile attaches one wait per producer
    semaphore. Hoist excess waits onto injected same-engine NOPs immediately
    before the instruction (engine queues are in-order, so waiting A then B
    sequentially == waiting {A,B} at once)."""
    import bass_rust
    import concourse.mybir as mybir

    n_split = 0
    for f in nc.m.functions:
        for bb in f.blocks:
            insts = bb.instructions
            out = []
            changed = False
            for inst in insts:
                si = inst.sync_info
                waits = list(si.on_wait) if si is not None and si.on_wait else []
                cap = 2 if isinstance(inst, mybir.InstEventSemaphore) else 1
                if len(waits) > cap:
                    changed = True
                    for w in waits[:-cap]:
                        nop = mybir.InstNoOp(
                            name=f"waitsplit_{n_split}", ins=[], outs=[]
                        )
                        n_split += 1
                        nop.engine = inst.engine
                        nop.sync_info = bass_rust.SyncInfo(on_wait=[w], on_update=[])
                        out.append(nop)
                    inst.sync_info = bass_rust.SyncInfo(
                        on_wait=waits[-cap:], on_update=si.on_update
                    )
                out.append(inst)
            if changed:
                insts[:] = out
                if bb.instructions[0].name != out[0].name or len(bb.instructions) != len(out):
                    raise RuntimeError("basic block instruction list not live-mutable")
    return n_split


def _rope_pair(nc, tmps, p0, p1, cos_s, sin_s, out0, out1):
    """out0 = p0*cos - p1*sin ; out1 = p1*cos + p0*sin  (RoPE half-pair).
    p0/p1: [128, L] f32 PSUM; copied to SBUF first (frees the PSUM bank
    after ~1 ACT op instead of after 4 DVE ops). cos/sin: [128, L] f32
    SBUF, out0/out1: [128, L] bf16 SBUF."""
    import concourse.mybir as mybir

    L = p0.shape[-1]
    c0 = tmps.tile([128, TQ], mybir.dt.float32, tag="projc")
    c1 = tmps.tile([128, TQ], mybir.dt.float32, tag="projc")
    nc.scalar.copy(c0[:, :L], p0)
    nc.vector.tensor_copy(c1[:, :L], p1)
    t0 = tmps.tile([128, TQ], mybir.dt.float32, tag="ropetmp")
    t1 = tmps.tile([128, TQ], mybir.dt.float32, tag="ropetmp")
    nc.vector.tensor_mul(t0[:, :L], c0[:, :L], cos_s)
    nc.vector.tensor_mul(t1[:, :L], c1[:, :L], sin_s)
    nc.vector.tensor_sub(out0, t0[:, :L], t1[:, :L])
    t2 = tmps.tile([128, TQ], mybir.dt.float32, tag="ropetmp")
    t3 = tmps.tile([128, TQ], mybir.dt.float32, tag="ropetmp")
    nc.vector.tensor_mul(t2[:, :L], c1[:, :L], cos_s)
    nc.vector.tensor_mul(t3[:, :L], c0[:, :L], sin_s)
    nc.vector.tensor_add(out1, t2[:, :L], t3[:, :L])


def _build_nc():
    import concourse.bass as bass
    import concourse.mybir as mybir
    import concourse.tile as tile

    F32 = mybir.dt.float32
    BF16 = mybir.dt.bfloat16
    EXP = mybir.ActivationFunctionType.Exp

    nc = bass.Bass()
    # All inputs are pre-arranged on the host to the exact SBUF layout so
    # every DMA moves >=2KB-contiguous per-partition lines at full rate.
    xt = nc.dram_tensor("xt", [NT, 128, NDC, TQ], BF16, kind="ExternalInput")
    qw = nc.dram_tensor("qw", [128, 2, NDC, H], BF16, kind="ExternalInput")
    kvw = nc.dram_tensor("kvw", [128, 2, NDC, H], BF16, kind="ExternalInput")
    outw = nc.dram_tensor("outw", [128, 2, 2, D], BF16, kind="ExternalInput")
    cost = nc.dram_tensor("cost", [HH, T], F32, kind="ExternalInput")
    sint = nc.dram_tensor("sint", [HH, T], F32, kind="ExternalInput")
    y = nc.dram_tensor("y", [T, D], BF16, kind="ExternalOutput")

    with tile.TileContext(nc) as tc:
        with (
            tc.tile_pool(name="const", bufs=1) as constp,
            tc.tile_pool(name="persist", bufs=1) as persist,
            tc.tile_pool(name="stream", bufs=2) as stream,
            tc.tile_pool(name="cstream", bufs=2) as cstream,
            tc.tile_pool(name="qtp", bufs=4) as qtp,
            tc.tile_pool(name="tmps", bufs=3) as tmps,
            tc.tile_pool(name="ptp", bufs=3) as ptp,
            tc.tile_pool(name="otp", bufs=2) as otp,
            tc.tile_pool(name="ysp", bufs=2) as ysp,
            tc.tile_pool(name="psum", bufs=1, space="PSUM") as psum,
        ):
            # --- constants -------------------------------------------------
            # Causal triangle (only the 128-wide diagonal block ever needs
            # masking): cmask[p, c] = 0 if c >= p else -1e30.
            cmask = constp.tile([128, 128], F32)
            nc.gpsimd.memset(cmask, 0.0)
            nc.gpsimd.affine_select(
                out=cmask,
                in_=cmask,
                compare_op=mybir.AluOpType.is_ge,
                fill=-1.0e30,
                base=0,
                pattern=[[1, 128]],
                channel_multiplier=-1,
            )
            ident = constp.tile([128, 128], BF16)
            from concourse.masks import make_identity

            make_identity(nc, ident)

            # HAM warm-up: the PE clock-gate defaults to half rate and takes
            # ~3.4us of sustained matmul activity to open. The first real
            # matmul waits a few us for the kvs/xt DMAs, so burn that window
            # on throwaway matmuls to enter the loop at full clock.
            warm = psum.tile([128, 128], F32, tag="projqk", bufs=2, name="warm")
            for _ in range(32):
                nc.tensor.matmul(warm, lhsT=ident, rhs=ident, start=True, stop=True)

            # --- resident weights / tables --------------------------------
            from concourse.tile import add_dep_helper

            # Startup DMA choreography: everything issued at t=0 shares HBM
            # bandwidth, so chain the DMAs pairwise in exactly the order the
            # PE consumes them: [kvs dg | x0 dg] pairs feed the K projection
            # chunk by chunk, then Q weights, V weights, out weights.
            kvs = constp.tile([128, 2, NDC, H], BF16)
            xts_t = [None] * NT
            xts_t[0] = stream.tile([128, NDC, TQ], BF16, tag="xts", name="xts")
            pair_last = []  # last DMA of the previous chained group
            d_crit = []
            d_x0 = []
            for dg in range(4):
                da = nc.sync.dma_start(
                    out=kvs[:, 0, 4 * dg : 4 * (dg + 1)],
                    in_=kvw[:, 0, 4 * dg : 4 * (dg + 1)],
                )
                db = nc.sync.dma_start(
                    out=xts_t[0][:, 4 * dg : 4 * (dg + 1), :],
                    in_=xt[0, :, 4 * dg : 4 * (dg + 1), :],
                )
                if dg >= 2:
                    # keep two groups in flight: group dg waits on dg-2
                    for dd in (da, db):
                        add_dep_helper(
                            dd.ins, d_x0[dg - 2].ins, reason="startup chain"
                        )
                d_crit.append(da)
                d_x0.append(db)
            css = [None] * NT
            sns = [None] * NT

            def cs_dma(j, deps=()):
                css[j] = cstream.tile([128, TQ], F32, tag="cs", name="cs")
                sns[j] = cstream.tile([128, TQ], F32, tag="sn", name="sn")
                jsl = slice(j * TQ, (j + 1) * TQ)
                d1 = nc.sync.dma_start(out=css[j], in_=cost[:, jsl])
                d2 = nc.sync.dma_start(out=sns[j], in_=sint[:, jsl])
                for dep in deps:
                    add_dep_helper(d1.ins, dep.ins, reason="defer cos/sin")
                    add_dep_helper(d2.ins, dep.ins, reason="defer cos/sin")

            cs_dma(0, deps=(d_x0[1],))
            qws = constp.tile([128, 2, NDC, H], BF16)
            d_qw = []
            for n in (0, 1):
                for hg in (0, 1):
                    dd = nc.sync.dma_start(
                        out=qws[:, n, 8 * hg : 8 * (hg + 1)],
                        in_=qw[:, n, 8 * hg : 8 * (hg + 1)],
                    )
                    # chain pairwise behind the K-proj stream
                    idx = 2 * n + hg
                    prev = d_x0[1 + idx] if idx < 3 else d_qw[0]
                    add_dep_helper(dd.ins, prev.ins, reason="startup chain")
                    d_qw.append(dd)
            d_kv1 = []
            for hg in (0, 1):
                dd = nc.sync.dma_start(
                    out=kvs[:, 1, 8 * hg : 8 * (hg + 1)],
                    in_=kvw[:, 1, 8 * hg : 8 * (hg + 1)],
                )
                add_dep_helper(dd.ins, d_qw[2 + hg].ins, reason="startup chain")
                d_kv1.append(dd)
            ows = constp.tile([128, 2, 2, D], BF16)
            for hg in (0, 1):
                dd = nc.sync.dma_start(out=ows[:, hg], in_=outw[:, hg])
                add_dep_helper(dd.ins, d_kv1[hg].ins, reason="startup chain")

            # K^T halves [h-half, t] and V chunks [s-in-chunk, h | ones],
            # grown per tile. The 257th column of each V chunk is constant 1.0
            # so the PV matmul accumulates the softmax denominator for free.
            VN = H + 1  # 257
            kts = persist.tile([128, 2, T], BF16)
            vs = persist.tile([128, T // 128, VN], BF16)
            nc.vector.memset(vs[:, :, H : H + 1], 1.0)

            for i in range(NT):
                tsl = slice(i * TQ, (i + 1) * TQ)
                cos_sl = css[i]
                sin_sl = sns[i]
                if i + 1 < NT:
                    cs_dma(i + 1)
                if i > 0:
                    # x^T slice [128, 16, 512] in 4 DMAs so the first
                    # projection matmuls can start on a quarter of the data
                    xts_t[i] = stream.tile(
                        [128, NDC, TQ], BF16, tag="xts", name="xts"
                    )
                    for dg in range(4):
                        nc.sync.dma_start(
                            out=xts_t[i][:, 4 * dg : 4 * (dg + 1), :],
                            in_=xt[i, :, 4 * dg : 4 * (dg + 1), :],
                        )
                xts = xts_t[i]

                # ---- K^T projection + RoPE -------------------------------
                ctx_proj = nc.named_scope(f"t{i}_proj"); ctx_proj.__enter__()
                kp0 = psum.tile([128, TQ], F32, tag="projqk", bufs=2)
                kp1 = psum.tile([128, TQ], F32, tag="projqk", bufs=2)
                for hh, kp in ((0, kp0), (1, kp1)):
                    for d in range(NDC):
                        nc.tensor.matmul(
                            kp,
                            lhsT=kvs[:, 0, d, hh * 128 : (hh + 1) * 128],
                            rhs=xts[:, d, :],
                            start=(d == 0),
                            stop=(d == NDC - 1),
                        )
                _rope_pair(
                    nc, tmps, kp0, kp1, cos_sl, sin_sl,
                    kts[:, 0, tsl], kts[:, 1, tsl],
                )

                # ---- Q^T projections + RoPE (2 heads) --------------------
                qt = []
                for n in (0, 1):
                    qp0 = psum.tile([128, TQ], F32, tag="projqk", bufs=2)
                    qp1 = psum.tile([128, TQ], F32, tag="projqk", bufs=2)
                    for hh, qp in ((0, qp0), (1, qp1)):
                        for d in range(NDC):
                            nc.tensor.matmul(
                                qp,
                                lhsT=qws[:, n, d, hh * 128 : (hh + 1) * 128],
                                rhs=xts[:, d, :],
                                start=(d == 0),
                                stop=(d == NDC - 1),
                            )
                    qtn = qtp.tile([128, 2, TQ], BF16, tag="qt")
                    _rope_pair(
                        nc, tmps, qp0, qp1, cos_sl, sin_sl,
                        qtn[:, 0, :], qtn[:, 1, :],
                    )
                    qt.append(qtn)

                # ---- V projection ----------------------------------------
                for ts in range(4):
                    vp = psum.tile([128, H], F32, tag="projqk", bufs=2)
                    for d in range(NDC):
                        nc.tensor.matmul(
                            vp,
                            lhsT=xts[:, d, ts * 128 : (ts + 1) * 128],
                            rhs=kvs[:, 1, d, :],
                            start=(d == 0),
                            stop=(d == NDC - 1),
                        )
                    nc.vector.tensor_copy(vs[:, 4 * i + ts, 0:H], vp)

                ctx_proj.__exit__(None, None, None)

                # ---- attention, software-pipelined -----------------------
                # O[t-sub, h|denom] accumulates per 128-row query sub-block in
                # PSUM over s-chunks: lhsT = P^T[s, t-sub], rhs = [V | 1].
                # The flat (head, chunk) step sequence runs QK(step j+1)
                # before PV(step j); normalization of sub-block ts is emitted
                # two steps after the chunk that finalizes it, and the output
                # projection for ts streams out right after head 1's ts.
                ctx_attn = nc.named_scope(f"t{i}_attn"); ctx_attn.__enter__()
                nchunks = 4 * i + 4
                ots = [
                    otp.tile([128, 2, TQ], BF16, tag=f"ot{n}", name=f"ot{n}")
                    for n in (0, 1)
                ]
                o_ps = {}
                norm_fifo = deque()
                state = {"pv": 0}

                def out_proj_ts(ts):
                    """Output projection for rows [i*TQ+ts*128, +128)."""
                    ys = ysp.tile([128, D], BF16, tag="ys", name="ys")
                    last = i == NT - 1 and ts == 3
                    for dc in range(4):
                        py = psum.tile(
                            [128, 512], F32, tag=f"o{ts}", name="py"
                        )
                        mm = 0
                        for n in (0, 1):
                            for hh in (0, 1):
                                nc.tensor.matmul(
                                    py,
                                    lhsT=ots[n][:, hh, ts * 128 : (ts + 1) * 128],
                                    rhs=ows[:, n, hh, dc * 512 : (dc + 1) * 512],
                                    start=(mm == 0),
                                    stop=(mm == 3),
                                )
                                mm += 1
                        if last:
                            # kernel tail: split the copy between DVE and ACT
                            # and DMA per 512-col chunk to shorten the drain
                            eng = nc.vector if dc % 2 == 0 else nc.scalar
                            if dc % 2 == 0:
                                eng.tensor_copy(
                                    ys[:, dc * 512 : (dc + 1) * 512], py
                                )
                            else:
                                eng.copy(ys[:, dc * 512 : (dc + 1) * 512], py)
                            nc.sync.dma_start(
                                out=y[
                                    i * TQ + ts * 128 : i * TQ + (ts + 1) * 128,
                                    dc * 512 : (dc + 1) * 512,
                                ],
                                in_=ys[:, dc * 512 : (dc + 1) * 512],
                            )
                        else:
                            nc.vector.tensor_copy(
                                ys[:, dc * 512 : (dc + 1) * 512], py
                            )
                    if not last:
                        nc.sync.dma_start(
                            out=y[i * TQ + ts * 128 : i * TQ + (ts + 1) * 128, :],
                            in_=ys,
                        )

                def emit_norm(n, ts):
                    """Normalize o_ps[n][ts] by its denominator column and
                    transpose into ots[n]; stream the output projection once
                    head 1's slice lands."""
                    tail = i == NT - 1 and ts >= 2
                    rd = tmps.tile([128, 1], F32, tag="rd", bufs=4)
                    nc.vector.reciprocal(rd, o_ps[n][ts][:, H : H + 1])
                    ob = tmps.tile([128, H], BF16, tag="ob", bufs=3)
                    if tail:
                        # kernel tail: ACT is idle, DVE is the critical chain
                        nc.scalar.mul(ob, o_ps[n][ts][:, 0:H], rd)
                    else:
                        nc.vector.tensor_scalar_mul(ob, o_ps[n][ts][:, 0:H], rd)
                    for hh in (0, 1):
                        tp = psum.tile(
                            [128, 128], BF16, tag="projqk", bufs=2, name="tp"
                        )
                        nc.tensor.transpose(
                            tp, ob[:, 128 * hh : 128 * (hh + 1)], ident
                        )
                        if tail:
                            nc.scalar.copy(
                                ots[n][:, hh, 128 * ts : 128 * (ts + 1)], tp
                            )
                        else:
                            nc.vector.tensor_copy(
                                ots[n][:, hh, 128 * ts : 128 * (ts + 1)], tp
                            )
                    if n == 1:
                        out_proj_ts(ts)

                def emit_pv(n, k, pt):
                    """PV matmuls for chunk k of head n, then any normalize
                    whose finalizing chunk's PV was emitted a step ago."""
                    if n not in o_ps:
                        o_ps[n] = [
                            psum.tile([128, VN], F32, tag=f"o{ts}", name=f"o{ts}")
                            for ts in range(4)
                        ]
                    q_ = max(0, k - 4 * i)
                    for ts in range(q_, 4):
                        nc.tensor.matmul(
                            o_ps[n][ts],
                            lhsT=pt[:, 128 * ts : 128 * (ts + 1)],
                            rhs=vs[:, k, :],
                            start=(k == 0),
                            stop=(k == 4 * i + ts),
                        )
                    state["pv"] += 1
                    if k >= 4 * i:
                        norm_fifo.append((n, k - 4 * i, state["pv"]))
                    while norm_fifo and norm_fifo[0][2] <= state["pv"] - 1:
                        nn, ts, _ = norm_fifo.popleft()
                        emit_norm(nn, ts)

                prev = None
                for n in (0, 1):
                    for k in range(nchunks):
                        q_ = max(0, k - 4 * i)
                        col0 = 128 * q_
                        ksl = slice(k * 128, (k + 1) * 128)
                        pl = psum.tile([128, TQ], F32, tag="work", bufs=2, name="pl")
                        nc.tensor.matmul(
                            pl[:, col0:],
                            lhsT=kts[:, 0, ksl],
                            rhs=qt[n][:, 0, col0:],
                            start=True,
                            stop=False,
                        )
                        nc.tensor.matmul(
                            pl[:, col0:],
                            lhsT=kts[:, 1, ksl],
                            rhs=qt[n][:, 1, col0:],
                            start=False,
                            stop=True,
                        )
                        if k >= 4 * i:
                            # only the 128-wide diagonal block needs masking
                            nc.vector.tensor_add(
                                pl[:, col0 : col0 + 128],
                                pl[:, col0 : col0 + 128],
                                cmask,
                            )
                        pt = ptp.tile([128, TQ], BF16, tag="pt", bufs=4, name="pt")
                        # exp split in two: PV's first LDWEIGHTS only needs
                        # the leading 128 columns, so publish those early
                        nc.scalar.activation(
                            pt[:, col0 : col0 + 128], pl[:, col0 : col0 + 128], EXP
                        )
                        if col0 + 128 < TQ:
                            nc.scalar.activation(
                                pt[:, col0 + 128 :], pl[:, col0 + 128 :], EXP
                            )
                        if prev is not None:
                            emit_pv(*prev)
                        prev = (n, k, pt)
                emit_pv(*prev)
                while norm_fifo:
                    nn, ts, _ = norm_fifo.popleft()
                    emit_norm(nn, ts)
                ctx_attn.__exit__(None, None, None)
    n = _split_excess_waits(nc)
    print(f"kernel build: split {n} excess waits")
    return nc


def _is_causal(mask):
    """mask: [B, T, T] bool — check it's exactly the causal tril mask."""
    tri = np.tril(np.ones((T, T), dtype=bool))
    return all(np.array_equal(mask[b], tri) for b in range(mask.shape[0]))


def _numpy_reference(x, segment_pos, attn_mask, q_w, kv_w, out_w):
    """Slow exact fallback for non-causal masks (matches reference.py)."""
    x = np.asarray(x, np.float32)
    out = np.zeros((B, T, D), np.float32)
    j = np.arange(HH, dtype=np.float32)
    timescale = 10000.0 ** (2.0 * j / H)
    for b in range(B):
        ang = segment_pos[b][:, None].astype(np.float32) / timescale[None, :]
        cos, sin = np.cos(ang), np.sin(ang)  # [T, 128]
        k = x[b] @ kv_w[0, 0]  # [T, H]
        v = x[b] @ kv_w[1, 0]
        k = np.concatenate(
            [k[:, :HH] * cos - k[:, HH:] * sin, k[:, HH:] * cos + k[:, :HH] * sin], 1
        )
        for n in range(N):
            q = x[b] @ q_w[n]
            q = np.concatenate(
                [q[:, :HH] * cos - q[:, HH:] * sin, q[:, HH:] * cos + q[:, :HH] * sin],
                1,
            ) * (H ** -0.5)
            logits = q @ k.T  # [T, T]
            logits = np.where(attn_mask[b], logits, -2.3819763e38)
            logits -= logits.max(-1, keepdims=True)
            p = np.exp(logits)
            p /= p.sum(-1, keepdims=True)
            out[b] += (p.astype(np.float32) @ v) @ out_w[n]
    return out


def kernel(x, segment_pos, attn_mask, q_w, kv_w, out_w):
    global LAST_RESULT
    x = np.asarray(x)
    segment_pos = np.asarray(segment_pos)
    attn_mask = np.asarray(attn_mask)
    q_w = np.asarray(q_w)
    kv_w = np.asarray(kv_w)
    out_w = np.asarray(out_w)
    assert x.shape == (B, T, D) and q_w.shape == (N, D, H)

    if not _is_causal(attn_mask):
        return _numpy_reference(x, segment_pos, attn_mask, q_w, kv_w, out_w)

    from concourse.bass_utils import run_bass_kernel_spmd

    if "nc" not in _CACHE:
        _CACHE["nc"] = _build_nc()
    nc = _CACHE["nc"]

    bf16 = ml_dtypes.bfloat16

    def dxh_pre(w):  # [2, D, H] -> [128, 2, NDC, H] (partition-major)
        return np.ascontiguousarray(
            w.reshape(2, NDC, 128, H).transpose(2, 0, 1, 3)
        ).astype(bf16)

    # Per-batch host prep
    xts, coss, sins = [], [], []
    j = np.arange(HH, dtype=np.float32)
    timescale = 10000.0 ** (2.0 * j / H)
    for b in range(B):
        # x[b] [T, D] -> x^T tiles [NT, 128(dp), NDC, TQ]
        xtp = np.ascontiguousarray(
            x[b].T.reshape(NDC, 128, NT, TQ).transpose(2, 1, 0, 3)
        ).astype(bf16)
        xts.append(xtp)
        ang = segment_pos[b][None, :].astype(np.float32) / timescale[:, None]
        coss.append(np.cos(ang).astype(np.float32))
        sins.append(np.sin(ang).astype(np.float32))
    kvw_host = dxh_pre(kv_w[:, 0])
    qw_scaled = q_w * np.float32(H ** -0.5)  # [N, D, H]
    # out_w [n, H, D] -> [128(hp), 2(n), 2(hh), D]
    outw_all = [
        np.ascontiguousarray(
            out_w[2 * m : 2 * m + 2].reshape(2, 2, 128, D).transpose(2, 0, 1, 3)
        ).astype(bf16)
        for m in range(4)
    ]
    qw_all = [dxh_pre(qw_scaled[2 * m : 2 * m + 2]) for m in range(4)]

    in_maps = []
    for c in range(NCORES):
        b, m = c // 4, c % 4
        in_maps.append(
            {
                "xt": xts[b],
                "qw": qw_all[m],
                "kvw": kvw_host,
                "outw": outw_all[m],
                "cost": coss[b],
                "sint": sins[b],
            }
        )

    trace = bool(int(os.environ.get("KERNEL_TRACE", "0")))
    res = run_bass_kernel_spmd(nc, in_maps, core_ids=list(range(NCORES)), trace=trace)
    LAST_RESULT = res

    out = np.zeros((B, T, D), np.float32)
    for c in range(NCORES):
        out[c // 4] += res.results[c]["y"].astype(np.float32)
    return out


# revision 21
# speedup vs baseline: 1.0558x; 1.0358x over previous
"""Trainium2 Bass kernel for GQA attention (B=2, T=4096, D=2048, N=8 q-heads,
K=1 kv-head, H=256) with RoPE + causal mask + output projection.

Sharding: data-parallel on batch (2) x tensor-parallel on query heads
(4 groups of 2 heads) = 8 cores. Each core computes a partial output
y_c = sum_{n in its 2 heads} softmax(q_n k^T) v @ out_w[n] for its batch;
the host sums the 4 partials per batch. (A cross-core AllReduce KV-dedup
was tried and reverted: collective SDMA traffic trips a GPIO power
throttle that caps the PE clock at 13/16 for most of the run, costing
more than the deduplicated projection work saved.)

The device kernel is identical on every core (single NEFF, SPMD); per-core
behaviour comes only from per-core input data:
  xt   [2048, 4096] bf16 : x[b]^T  (pre-transposed + bf16 on host)
  qw   [2, 2048, 256] bf16 : q_w for the core's 2 heads, pre-scaled by H^-0.5
  kvw  [2, 2048, 256] bf16 : k/v projection weights (shared kv head)
  outw [2, 256, 2048] bf16 : out_w for the core's 2 heads
  cost/sint [128, 4096] f32 : RoPE cos/sin tables (timescale j x position t)
Output: y [4096, 2048] bf16 partial (summed in f32 on host).

Flash-attention layout: everything transposed (S^T = K^T^T-contraction) so
softmax statistics land in matmuls:
  K^T,Q^T [h, t] from projections directly; logits S^T [s-chunk 128, t 512]
  in PSUM; exp on ACT -> P^T bf16; PV as pt-stationary matmul giving
  O [t-sub, h | denom] accumulated over s-chunks in PSUM; denominator via
  a constant-1 column appended to V; normalization by per-partition DVE
  scale, then PE transpose to O^T for the output projection.

Scheduling: the (head, chunk) loop is software-pipelined one step deep --
QK(k+1) is issued on the PE before PV(k) -- so the QK->exp->PV chain
latency (ACT engine) is hidden behind the next chunk's QK matmuls.
Normalization of query sub-block ts is issued 2 steps after the chunk that
finalizes its PSUM row, and the output projection streams out per 128-row
sub-block as soon as both heads' normalized O^T slices exist.
"""

import os
from collections import deque

import numpy as np
import ml_dtypes

B, T, D, N, H = 2, 4096, 2048, 8, 256
NCORES = 8
HH = H // 2  # 128, also the RoPE pair offset and partition size
TQ = 512     # query-tile columns (moving dim of logits matmul)
NT = T // TQ # 8 query tiles
NDC = D // 128  # 16 contraction chunks over D

_CACHE = {}
LAST_RESULT = None  # BassKernelResults of the most recent device run (for test harness)


def _split_excess_waits(nc):
    """The walrus in this container accepts at most 1 sync-wait per
    instruction (2 for EventSemaphore); Tile attaches one wait per producer
    semaphore. Hoist excess waits onto injected same-engine NOPs immediately
    before the instruction (engine queues are in-order, so waiting A then B
    sequentially == waiting {A,B} at once)."""
    import bass_rust
    import concourse.mybir as mybir

    n_split = 0
    for f in nc.m.functions:
        for bb in f.blocks:
            insts = bb.instructions
            out = []
            changed = False
            for inst in insts:
                si = inst.sync_info
                waits = list(si.on_wait) if si is not None and si.on_wait else []
                cap = 2 if isinstance(inst, mybir.InstEventSemaphore) else 1
                if len(waits) > cap:
                    changed = True
                    for w in waits[:-cap]:
                        nop = mybir.InstNoOp(
                            name=f"waitsplit_{n_split}", ins=[], outs=[]
                        )
                        n_split += 1
                        nop.engine = inst.engine
                        nop.sync_info = bass_rust.SyncInfo(on_wait=[w], on_update=[])
                        out.append(nop)
                    inst.sync_info = bass_rust.SyncInfo(
                        on_wait=waits[-cap:], on_update=si.on_update
                    )
                out.append(inst)
            if changed:
                insts[:] = out
                if bb.instructions[0].name != out[0].name or len(bb.instructions) != len(out):
                    raise RuntimeError("basic block instruction list not live-mutable")
    return n_split


def _rope_pair(nc, tmps, p0, p1, cos_s, sin_s, out0, out1):
    """out0 = p0*cos - p1*sin ; out1 = p1*cos + p0*sin  (RoPE half-pair).
    p0/p1: [128, L] f32 PSUM; copied to SBUF first (frees the PSUM bank
    after ~1 ACT op instead of after 4 DVE ops). cos/sin: [128, L] f32
    SBUF, out0/out1: [128, L] bf16 SBUF."""
    import concourse.mybir as mybir

    L = p0.shape[-1]
    c0 = tmps.tile([128, TQ], mybir.dt.float32, tag="projc")
    c1 = tmps.tile([128, TQ], mybir.dt.float32, tag="projc")
    nc.scalar.copy(c0[:, :L], p0)
    nc.vector.tensor_copy(c1[:, :L], p1)
    t0 = tmps.tile([128, TQ], mybir.dt.float32, tag="ropetmp")
    t1 = tmps.tile([128, TQ], mybir.dt.float32, tag="ropetmp")
    nc.vector.tensor_mul(t0[:, :L], c0[:, :L], cos_s)
    nc.vector.tensor_mul(t1[:, :L], c1[:, :L], sin_s)
    nc.vector.tensor_sub(out0, t0[:, :L], t1[:, :L])
    t2 = tmps.tile([128, TQ], mybir.dt.float32, tag="ropetmp")
    t3 = tmps.tile([128, TQ], mybir.dt.float32, tag="ropetmp")
    nc.vector.tensor_mul(t2[:, :L], c1[:, :L], cos_s)
    nc.vector.tensor_mul(t3[:, :L], c0[:, :L], sin_s)
    nc.vector.tensor_add(out1, t2[:, :L], t3[:, :L])


def _build_nc():
    import concourse.bass as bass
    import concourse.mybir as mybir
    import concourse.tile as tile

    F32 = mybir.dt.float32
    BF16 = mybir.dt.bfloat16
    EXP = mybir.ActivationFunctionType.Exp

    nc = bass.Bass()
    # All inputs are pre-arranged on the host to the exact SBUF layout so
    # every DMA moves >=2KB-contiguous per-partition lines at full rate.
    xt = nc.dram_tensor("xt", [NT, 128, NDC, TQ], BF16, kind="ExternalInput")
    qw = nc.dram_tensor("qw", [128, 2, NDC, H], BF16, kind="ExternalInput")
    kvw = nc.dram_tensor("kvw", [128, 2, NDC, H], BF16, kind="ExternalInput")
    outw = nc.dram_tensor("outw", [128, 2, 2, D], BF16, kind="ExternalInput")
    cost = nc.dram_tensor("cost", [HH, T], F32, kind="ExternalInput")
    sint = nc.dram_tensor("sint", [HH, T], F32, kind="ExternalInput")
    y = nc.dram_tensor("y", [T, D], BF16, kind="ExternalOutput")

    with tile.TileContext(nc) as tc:
        with (
            tc.tile_pool(name="const", bufs=1) as constp,
            tc.tile_pool(name="persist", bufs=1) as persist,
            tc.tile_pool(name="stream", bufs=2) as stream,
            tc.tile_pool(name="cstream", bufs=2) as cstream,
            tc.tile_pool(name="qtp", bufs=4) as qtp,
            tc.tile_pool(name="tmps", bufs=3) as tmps,
            tc.tile_pool(name="ptp", bufs=3) as ptp,
            tc.tile_pool(name="otp", bufs=2) as otp,
            tc.tile_pool(name="ysp", bufs=2) as ysp,
            tc.tile_pool(name="psum", bufs=1, space="PSUM") as psum,
        ):
            # --- constants -------------------------------------------------
            # Causal triangle (only the 128-wide diagonal block ever needs
            # masking): cmask[p, c] = 0 if c >= p else -1e30.
            cmask = constp.tile([128, 128], F32)
            nc.gpsimd.memset(cmask, 0.0)
            nc.gpsimd.affine_select(
                out=cmask,
                in_=cmask,
                compare_op=mybir.AluOpType.is_ge,
                fill=-1.0e30,
                base=0,
                pattern=[[1, 128]],
                channel_multiplier=-1,
            )
            ident = constp.tile([128, 128], BF16)
            from concourse.masks import make_identity

            make_identity(nc, ident)

            # HAM warm-up: the PE clock-gate defaults to half rate and takes
            # ~3.4us of sustained matmul activity to open. The first real
            # matmul waits a few us for the kvs/xt DMAs, so burn that window
            # on throwaway matmuls to enter the loop at full clock.
            warm = psum.tile([128, 128], F32, tag="projqk", bufs=2, name="warm")
            for _ in range(32):
                nc.tensor.matmul(warm, lhsT=ident, rhs=ident, start=True, stop=True)

            # --- resident weights / tables --------------------------------
            from concourse.tile import add_dep_helper

            # Startup DMA choreography: everything issued at t=0 shares HBM
            # bandwidth, so chain the DMAs pairwise in exactly the order the
            # PE consumes them: [kvs dg | x0 dg] pairs feed the K projection
            # chunk by chunk, then Q weights, V weights, out weights.
            kvs = constp.tile([128, 2, NDC, H], BF16)
            xts_t = [None] * NT
            xts_t[0] = stream.tile([128, NDC, TQ], BF16, tag="xts", name="xts")
            pair_last = []  # last DMA of the previous chained group
            d_crit = []
            d_x0 = []
            for dg in range(4):
                da = nc.sync.dma_start(
                    out=kvs[:, 0, 4 * dg : 4 * (dg + 1)],
                    in_=kvw[:, 0, 4 * dg : 4 * (dg + 1)],
                )
                db = nc.sync.dma_start(
                    out=xts_t[0][:, 4 * dg : 4 * (dg + 1), :],
                    in_=xt[0, :, 4 * dg : 4 * (dg + 1), :],
                )
                if dg >= 2:
                    # keep two groups in flight: group dg waits on dg-2
                    for dd in (da, db):
                        add_dep_helper(
                            dd.ins, d_x0[dg - 2].ins, reason="startup chain"
                        )
                d_crit.append(da)
                d_x0.append(db)
            css = [None] * NT
            sns = [None] * NT

            def cs_dma(j, deps=()):
                css[j] = cstream.tile([128, TQ], F32, tag="cs", name="cs")
                sns[j] = cstream.tile([128, TQ], F32, tag="sn", name="sn")
                jsl = slice(j * TQ, (j + 1) * TQ)
                d1 = nc.sync.dma_start(out=css[j], in_=cost[:, jsl])
                d2 = nc.sync.dma_start(out=sns[j], in_=sint[:, jsl])
                for dep in deps:
                    add_dep_helper(d1.ins, dep.ins, reason="defer cos/sin")
                    add_dep_helper(d2.ins, dep.ins, reason="defer cos/sin")

            cs_dma(0, deps=(d_x0[1],))
            qws = constp.tile([128, 2, NDC, H], BF16)
            d_qw = []
            for n in (0, 1):
                for hg in (0, 1):
                    dd = nc.sync.dma_start(
                        out=qws[:, n, 8 * hg : 8 * (hg + 1)],
                        in_=qw[:, n, 8 * hg : 8 * (hg + 1)],
                    )
                    # chain pairwise behind the K-proj stream
                    idx = 2 * n + hg
                    prev = d_x0[1 + idx] if idx < 3 else d_qw[0]
                    add_dep_helper(dd.ins, prev.ins, reason="startup chain")
                    d_qw.append(dd)
            d_kv1 = []
            for hg in (0, 1):
                dd = nc.sync.dma_start(
                    out=kvs[:, 1, 8 * hg : 8 * (hg + 1)],
                    in_=kvw[:, 1, 8 * hg : 8 * (hg + 1)],
                )
                add_dep_helper(dd.ins, d_qw[2 + hg].ins, reason="startup chain")
                d_kv1.append(dd)
            ows = constp.tile([128, 2, 2, D], BF16)
            for hg in (0, 1):
                dd = nc.sync.dma_start(out=ows[:, hg], in_=outw[:, hg])
                add_dep_helper(dd.ins, d_kv1[hg].ins, reason="startup chain")

            # K^T halves [h-half, t] and V chunks [s-in-chunk, h | ones],
            # grown per tile. The 257th column of each V chunk is constant 1.0
            # so the PV matmul accumulates the softmax denominator for free.
            VN = H + 1  # 257
            kts = persist.tile([128, 2, T], BF16)
            vs = persist.tile([128, T // 128, VN], BF16)
            nc.vector.memset(vs[:, :, H : H + 1], 1.0)

            for i in range(NT):
                tsl = slice(i * TQ, (i + 1) * TQ)
                cos_sl = css[i]
                sin_sl = sns[i]
                if i + 1 < NT:
                    cs_dma(i + 1)
                if i > 0:
                    # x^T slice [128, 16, 512] in 4 DMAs so the first
                    # projection matmuls can start on a quarter of the data
                    xts_t[i] = stream.tile(
                        [128, NDC, TQ], BF16, tag="xts", name="xts"
                    )
                    for dg in range(4):
                        nc.sync.dma_start(
                            out=xts_t[i][:, 4 * dg : 4 * (dg + 1), :],
                            in_=xt[i, :, 4 * dg : 4 * (dg + 1), :],
                        )
                xts = xts_t[i]

                # ---- K^T projection + RoPE -------------------------------
                ctx_proj = nc.named_scope(f"t{i}_proj"); ctx_proj.__enter__()
                kp0 = psum.tile([128, TQ], F32, tag="projqk", bufs=2)
                kp1 = psum.tile([128, TQ], F32, tag="projqk", bufs=2)
                for hh, kp in ((0, kp0), (1, kp1)):
                    for d in range(NDC):
                        nc.tensor.matmul(
                            kp,
                            lhsT=kvs[:, 0, d, hh * 128 : (hh + 1) * 128],
                            rhs=xts[:, d, :],
                            start=(d == 0),
                            stop=(d == NDC - 1),
                        )
                _rope_pair(
                    nc, tmps, kp0, kp1, cos_sl, sin_sl,
                    kts[:, 0, tsl], kts[:, 1, tsl],
                )

                # ---- Q^T projections + RoPE (2 heads) --------------------
                qt = []
                for n in (0, 1):
                    qp0 = psum.tile([128, TQ], F32, tag="projqk", bufs=2)
                    qp1 = psum.tile([128, TQ], F32, tag="projqk", bufs=2)
                    for hh, qp in ((0, qp0), (1, qp1)):
                        for d in range(NDC):
                            nc.tensor.matmul(
                                qp,
                                lhsT=qws[:, n, d, hh * 128 : (hh + 1) * 128],
                                rhs=xts[:, d, :],
                                start=(d == 0),
                                stop=(d == NDC - 1),
                            )
                    qtn = qtp.tile([128, 2, TQ], BF16, tag="qt")
                    _rope_pair(
                        nc, tmps, qp0, qp1, cos_sl, sin_sl,
                        qtn[:, 0, :], qtn[:, 1, :],
                    )
                    qt.append(qtn)

                # ---- V projection ----------------------------------------
                for ts in range(4):
                    vp = psum.tile([128, H], F32, tag="projqk", bufs=2)
                    for d in range(NDC):
                        nc.tensor.matmul(
                            vp,
                            lhsT=xts[:, d, ts * 128 : (ts + 1) * 128],
                            rhs=kvs[:, 1, d, :],
                            start=(d == 0),
                            stop=(d == NDC - 1),
                        )
                    nc.vector.tensor_copy(vs[:, 4 * i + ts, 0:H], vp)

                ctx_proj.__exit__(None, None, None)

                # ---- attention, software-pipelined -----------------------
                # O[t-sub, h|denom] accumulates per 128-row query sub-block in
                # PSUM over s-chunks: lhsT = P^T[s, t-sub], rhs = [V | 1].
                # The flat (head, chunk) step sequence runs QK(step j+1)
                # before PV(step j); normalization of sub-block ts is emitted
                # two steps after the chunk that finalizes it, and the output
                # projection for ts streams out right after head 1's ts.
                ctx_attn = nc.named_scope(f"t{i}_attn"); ctx_attn.__enter__()
                nchunks = 4 * i + 4
                ots = [
                    otp.tile([128, 2, TQ], BF16, tag=f"ot{n}", name=f"ot{n}")
                    for n in (0, 1)
                ]
                o_ps = {}
                norm_fifo = deque()
                state = {"pv": 0}

                def out_proj_ts(ts):
                    """Output projection for rows [i*TQ+ts*128, +128)."""
                    ys = ysp.tile([128, D], BF16, tag="ys", name="ys")
                    last = i == NT - 1 and ts == 3
                    for dc in range(4):
                        py = psum.tile(
                            [128, 512], F32, tag=f"o{ts}", name="py"
                        )
                        mm = 0
                        for n in (0, 1):
                            for hh in (0, 1):
                                nc.tensor.matmul(
                                    py,
                                    lhsT=ots[n][:, hh, ts * 128 : (ts + 1) * 128],
                                    rhs=ows[:, n, hh, dc * 512 : (dc + 1) * 512],
                                    start=(mm == 0),
                                    stop=(mm == 3),
                                )
                                mm += 1
                        if last:
                            # kernel tail: split the copy between DVE and ACT
                            # and DMA per 512-col chunk to shorten the drain
                            eng = nc.vector if dc % 2 == 0 else nc.scalar
                            if dc % 2 == 0:
                                eng.tensor_copy(
                                    ys[:, dc * 512 : (dc + 1) * 512], py
                                )
                            else:
                                eng.copy(ys[:, dc * 512 : (dc + 1) * 512], py)
                            nc.sync.dma_start(
                                out=y[
                                    i * TQ + ts * 128 : i * TQ + (ts + 1) * 128,
                                    dc * 512 : (dc + 1) * 512,
                                ],
                                in_=ys[:, dc * 512 : (dc + 1) * 512],
                            )
                        else:
                            nc.vector.tensor_copy(
                                ys[:, dc * 512 : (dc + 1) * 512], py
                            )
                    if not last:
                        nc.sync.dma_start(
                            out=y[i * TQ + ts * 128 : i * TQ + (ts + 1) * 128, :],
                            in_=ys,
                        )

                def emit_norm(n, ts):
                    """Normalize o_ps[n][ts] by its denominator column and
                    transpose into ots[n]; stream the output projection once
                    head 1's slice lands."""
                    tail = i == NT - 1 and ts >= 2
                    rd = tmps.tile([128, 1], F32, tag="rd", bufs=4)
                    nc.vector.reciprocal(rd, o_ps[n][ts][:, H : H + 1])
                    ob = tmps.tile([128, H], BF16, tag="ob", bufs=3)
                    if tail:
                        # kernel tail: ACT is idle, DVE is the critical chain
                        nc.scalar.mul(ob, o_ps[n][ts][:, 0:H], rd)
                    else:
                        nc.vector.tensor_scalar_mul(ob, o_ps[n][ts][:, 0:H], rd)
                    for hh in (0, 1):
                        tp = psum.tile(
                            [128, 128], BF16, tag="projqk", bufs=2, name="tp"
                        )
                        nc.tensor.transpose(
                            tp, ob[:, 128 * hh : 128 * (hh + 1)], ident
                        )
                        if tail:
                            nc.scalar.copy(
                                ots[n][:, hh, 128 * ts : 128 * (ts + 1)], tp
                            )
                        else:
                            nc.vector.tensor_copy(
                                ots[n][:, hh, 128 * ts : 128 * (ts + 1)], tp
                            )
                    if n == 1:
                        out_proj_ts(ts)

                def emit_pv(n, k, pt):
                    """PV matmuls for chunk k of head n, then any normalize
                    whose finalizing chunk's PV was emitted a step ago."""
                    if n not in o_ps:
                        o_ps[n] = [
                            psum.tile([128, VN], F32, tag=f"o{ts}", name=f"o{ts}")
                            for ts in range(4)
                        ]
                    q_ = max(0, k - 4 * i)
                    for ts in range(q_, 4):
                        nc.tensor.matmul(
                            o_ps[n][ts],
                            lhsT=pt[:, 128 * ts : 128 * (ts + 1)],
                            rhs=vs[:, k, :],
                            start=(k == 0),
                            stop=(k == 4 * i + ts),
                        )
                    state["pv"] += 1
                    if k >= 4 * i:
                        norm_fifo.append((n, k - 4 * i, state["pv"]))
                    while norm_fifo and norm_fifo[0][2] <= state["pv"] - 1:
                        nn, ts, _ = norm_fifo.popleft()
                        emit_norm(nn, ts)

                prev = None
                for n in (0, 1):
                    for k in range(nchunks):
                        q_ = max(0, k - 4 * i)
                        col0 = 128 * q_
                        ksl = slice(k * 128, (k + 1) * 128)
                        pl = psum.tile([128, TQ], F32, tag="work", bufs=2, name="pl")
                        nc.tensor.matmul(
                            pl[:, col0:],
                            lhsT=kts[:, 0, ksl],
                            rhs=qt[n][:, 0, col0:],
                            start=True,
                            stop=False,
                        )
                        nc.tensor.matmul(
                            pl[:, col0:],
                            lhsT=kts[:, 1, ksl],
                            rhs=qt[n][:, 1, col0:],
                            start=False,
                            stop=True,
                        )
                        if k >= 4 * i:
                            # only the 128-wide diagonal block needs masking
                            nc.vector.tensor_add(
                                pl[:, col0 : col0 + 128],
                                pl[:, col0 : col0 + 128],
                                cmask,
                            )
                        pt = ptp.tile([128, TQ], BF16, tag="pt", bufs=4, name="pt")
                        nc.scalar.activation(pt[:, col0:], pl[:, col0:], EXP)
                        if prev is not None:
                            emit_pv(*prev)
                        prev = (n, k, pt)
                emit_pv(*prev)
                while norm_fifo:
                    nn, ts, _ = norm_fifo.popleft()
                    emit_norm(nn, ts)
                ctx_attn.__exit__(None, None, None)
    n = _split_excess_waits(nc)
    print(f"kernel build: split {n} excess waits")
    return nc


def _is_causal(mask):
    """mask: [B, T, T] bool — check it's exactly the causal tril mask."""
    tri = np.tril(np.ones((T, T), dtype=bool))
    return all(np.array_equal(mask[b], tri) for b in range(mask.shape[0]))


def _numpy_reference(x, segment_pos, attn_mask, q_w, kv_w, out_w):
    """Slow exact fallback for non-causal masks (matches reference.py)."""
    x = np.asarray(x, np.float32)
    out = np.zeros((B, T, D), np.float32)
    j = np.arange(HH, dtype=np.float32)
    timescale = 10000.0 ** (2.0 * j / H)
    for b in range(B):
        ang = segment_pos[b][:, None].astype(np.float32) / timescale[None, :]
        cos, sin = np.cos(ang), np.sin(ang)  # [T, 128]
        k = x[b] @ kv_w[0, 0]  # [T, H]
        v = x[b] @ kv_w[1, 0]
        k = np.concatenate(
            [k[:, :HH] * cos - k[:, HH:] * sin, k[:, HH:] * cos + k[:, :HH] * sin], 1
        )
        for n in range(N):
            q = x[b] @ q_w[n]
            q = np.concatenate(
                [q[:, :HH] * cos - q[:, HH:] * sin, q[:, HH:] * cos + q[:, :HH] * sin],
                1,
            ) * (H ** -0.5)
            logits = q @ k.T  # [T, T]
            logits = np.where(attn_mask[b], logits, -2.3819763e38)
            logits -= logits.max(-1, keepdims=True)
            p = np.exp(logits)
            p /= p.sum(-1, keepdims=True)
            out[b] += (p.astype(np.float32) @ v) @ out_w[n]
    return out


def kernel(x, segment_pos, attn_mask, q_w, kv_w, out_w):
    global LAST_RESULT
    x = np.asarray(x)
    segment_pos = np.asarray(segment_pos)
    attn_mask = np.asarray(attn_mask)
    q_w = np.asarray(q_w)
    kv_w = np.asarray(kv_w)
    out_w = np.asarray(out_w)
    assert x.shape == (B, T, D) and q_w.shape == (N, D, H)

    if not _is_causal(attn_mask):
        return _numpy_reference(x, segment_pos, attn_mask, q_w, kv_w, out_w)

    from concourse.bass_utils import run_bass_kernel_spmd

    if "nc" not in _CACHE:
        _CACHE["nc"] = _build_nc()
    nc = _CACHE["nc"]

    bf16 = ml_dtypes.bfloat16

    def dxh_pre(w):  # [2, D, H] -> [128, 2, NDC, H] (partition-major)
        return np.ascontiguousarray(
            w.reshape(2, NDC, 128, H).transpose(2, 0, 1, 3)
        ).astype(bf16)

    # Per-batch host prep
    xts, coss, sins = [], [], []
    j = np.arange(HH, dtype=np.float32)
    timescale = 10000.0 ** (2.0 * j / H)
    for b in range(B):
        # x[b] [T, D] -> x^T tiles [NT, 128(dp), NDC, TQ]
        xtp = np.ascontiguousarray(
            x[b].T.reshape(NDC, 128, NT, TQ).transpose(2, 1, 0, 3)
        ).astype(bf16)
        xts.append(xtp)
        ang = segment_pos[b][None, :].astype(np.float32) / timescale[:, None]
        coss.append(np.cos(ang).astype(np.float32))
        sins.append(np.sin(ang).astype(np.float32))
    kvw_host = dxh_pre(kv_w[:, 0])
    qw_scaled = q_w * np.float32(H ** -0.5)  # [N, D, H]
    # out_w [n, H, D] -> [128(hp), 2(n), 2(hh), D]
    outw_all = [
        np.ascontiguousarray(
            out_w[2 * m : 2 * m + 2].reshape(2, 2, 128, D).transpose(2, 0, 1, 3)
        ).astype(bf16)
        for m in range(4)
    ]
    qw_all = [dxh_pre(qw_scaled[2 * m : 2 * m + 2]) for m in range(4)]

    in_maps = []
    for c in range(NCORES):
        b, m = c // 4, c % 4
        in_maps.append(
            {
                "xt": xts[b],
                "qw": qw_all[m],
                "kvw": kvw_host,
                "outw": outw_all[m],
                "cost": coss[b],
                "sint": sins[b],
            }
        )

    trace = bool(int(os.environ.get("KERNEL_TRACE", "0")))
    res = run_bass_kernel_spmd(nc, in_maps, core_ids=list(range(NCORES)), trace=trace)
    LAST_RESULT = res

    out = np.zeros((B, T, D), np.float32)
    for c in range(NCORES):
        out[c // 4] += res.results[c]["y"].astype(np.float32)
    return out


# revision 24
# speedup vs baseline: 1.0574x; 1.0014x over previous
"""Trainium2 Bass kernel for GQA attention (B=2, T=4096, D=2048, N=8 q-heads,
K=1 kv-head, H=256) with RoPE + causal mask + output projection.

Sharding: data-parallel on batch (2) x tensor-parallel on query heads
(4 groups of 2 heads) = 8 cores. Each core computes a partial output
y_c = sum_{n in its 2 heads} softmax(q_n k^T) v @ out_w[n] for its batch;
the host sums the 4 partials per batch. (A cross-core AllReduce KV-dedup
was tried and reverted: collective SDMA traffic trips a GPIO power
throttle that caps the PE clock at 13/16 for most of the run, costing
more than the deduplicated projection work saved.)

The device kernel is identical on every core (single NEFF, SPMD); per-core
behaviour comes only from per-core input data:
  xt   [2048, 4096] bf16 : x[b]^T  (pre-transposed + bf16 on host)
  qw   [2, 2048, 256] bf16 : q_w for the core's 2 heads, pre-scaled by H^-0.5
  kvw  [2, 2048, 256] bf16 : k/v projection weights (shared kv head)
  outw [2, 256, 2048] bf16 : out_w for the core's 2 heads
  cost/sint [128, 4096] f32 : RoPE cos/sin tables (timescale j x position t)
Output: y [4096, 2048] bf16 partial (summed in f32 on host).

Flash-attention layout: everything transposed (S^T = K^T^T-contraction) so
softmax statistics land in matmuls:
  K^T,Q^T [h, t] from projections directly; logits S^T [s-chunk 128, t 512]
  in PSUM; exp on ACT -> P^T bf16; PV as pt-stationary matmul giving
  O [t-sub, h | denom] accumulated over s-chunks in PSUM; denominator via
  a constant-1 column appended to V; normalization by per-partition DVE
  scale, then PE transpose to O^T for the output projection.

Scheduling: the (head, chunk) loop is software-pipelined one step deep --
QK(k+1) is issued on the PE before PV(k) -- so the QK->exp->PV chain
latency (ACT engine) is hidden behind the next chunk's QK matmuls.
Normalization of query sub-block ts is issued 2 steps after the chunk that
finalizes its PSUM row, and the output projection streams out per 128-row
sub-block as soon as both heads' normalized O^T slices exist.
"""

import os
from collections import deque

import numpy as np
import ml_dtypes

B, T, D, N, H = 2, 4096, 2048, 8, 256
NCORES = 8
HH = H // 2  # 128, also the RoPE pair offset and partition size
TQ = 512     # query-tile columns (moving dim of logits matmul)
NT = T // TQ # 8 query tiles
NDC = D // 128  # 16 contraction chunks over D

_CACHE = {}
LAST_RESULT = None  # BassKernelResults of the most recent device run (for test harness)


def _split_excess_waits(nc):
    """The walrus in this container accepts at most 1 sync-wait per
    instruction (2 for EventSemaphore); Tile attaches one wait per producer
    semaphore. Hoist excess waits onto injected same-engine NOPs immediately
    before the instruction (engine queues are in-order, so waiting A then B
    sequentially == waiting {A,B} at once)."""
    import bass_rust
    import concourse.mybir as mybir

    n_split = 0
    for f in nc.m.functions:
        for bb in f.blocks:
            insts = bb.instructions
            out = []
            changed = False
            for inst in insts:
                si = inst.sync_info
                waits = list(si.on_wait) if si is not None and si.on_wait else []
                cap = 2 if isinstance(inst, mybir.InstEventSemaphore) else 1
                if len(waits) > cap:
                    changed = True
                    for w in waits[:-cap]:
                        nop = mybir.InstNoOp(
                            name=f"waitsplit_{n_split}", ins=[], outs=[]
                        )
                        n_split += 1
                        nop.engine = inst.engine
                        nop.sync_info = bass_rust.SyncInfo(on_wait=[w], on_update=[])
                        out.append(nop)
                    inst.sync_info = bass_rust.SyncInfo(
                        on_wait=waits[-cap:], on_update=si.on_update
                    )
                out.append(inst)
            if changed:
                insts[:] = out
                if bb.instructions[0].name != out[0].name or len(bb.instructions) != len(out):
                    raise RuntimeError("basic block instruction list not live-mutable")
    return n_split


def _rope_pair(nc, tmps, p0, p1, cos_s, sin_s, out0, out1):
    """out0 = p0*cos - p1*sin ; out1 = p1*cos + p0*sin  (RoPE half-pair).
    p0/p1: [128, L] f32 PSUM; copied to SBUF first (frees the PSUM bank
    after ~1 ACT op instead of after 4 DVE ops). cos/sin: [128, L] f32
    SBUF, out0/out1: [128, L] bf16 SBUF."""
    import concourse.mybir as mybir

    L = p0.shape[-1]
    c0 = tmps.tile([128, TQ], mybir.dt.float32, tag="projc")
    c1 = tmps.tile([128, TQ], mybir.dt.float32, tag="projc")
    nc.scalar.copy(c0[:, :L], p0)
    nc.vector.tensor_copy(c1[:, :L], p1)
    t0 = tmps.tile([128, TQ], mybir.dt.float32, tag="ropetmp")
    t1 = tmps.tile([128, TQ], mybir.dt.float32, tag="ropetmp")
    nc.vector.tensor_mul(t0[:, :L], c0[:, :L], cos_s)
    nc.vector.tensor_mul(t1[:, :L], c1[:, :L], sin_s)
    nc.vector.tensor_sub(out0, t0[:, :L], t1[:, :L])
    t2 = tmps.tile([128, TQ], mybir.dt.float32, tag="ropetmp")
    t3 = tmps.tile([128, TQ], mybir.dt.float32, tag="ropetmp")
    nc.vector.tensor_mul(t2[:, :L], c1[:, :L], cos_s)
    nc.vector.tensor_mul(t3[:, :L], c0[:, :L], sin_s)
    nc.vector.tensor_add(out1, t2[:, :L], t3[:, :L])


def _build_nc():
    import concourse.bass as bass
    import concourse.mybir as mybir
    import concourse.tile as tile

    F32 = mybir.dt.float32
    BF16 = mybir.dt.bfloat16
    EXP = mybir.ActivationFunctionType.Exp

    nc = bass.Bass()
    # All inputs are pre-arranged on the host to the exact SBUF layout so
    # every DMA moves >=2KB-contiguous per-partition lines at full rate.
    xt = nc.dram_tensor("xt", [NT, 128, NDC, TQ], BF16, kind="ExternalInput")
    qw = nc.dram_tensor("qw", [128, 2, NDC, H], BF16, kind="ExternalInput")
    kvw = nc.dram_tensor("kvw", [128, 2, NDC, H], BF16, kind="ExternalInput")
    outw = nc.dram_tensor("outw", [128, 2, 2, D], BF16, kind="ExternalInput")
    cost = nc.dram_tensor("cost", [HH, T], F32, kind="ExternalInput")
    sint = nc.dram_tensor("sint", [HH, T], F32, kind="ExternalInput")
    y = nc.dram_tensor("y", [T, D], BF16, kind="ExternalOutput")

    with tile.TileContext(nc) as tc:
        with (
            tc.tile_pool(name="const", bufs=1) as constp,
            tc.tile_pool(name="persist", bufs=1) as persist,
            tc.tile_pool(name="stream", bufs=2) as stream,
            tc.tile_pool(name="cstream", bufs=2) as cstream,
            tc.tile_pool(name="qtp", bufs=4) as qtp,
            tc.tile_pool(name="tmps", bufs=3) as tmps,
            tc.tile_pool(name="ptp", bufs=3) as ptp,
            tc.tile_pool(name="otp", bufs=2) as otp,
            tc.tile_pool(name="ysp", bufs=2) as ysp,
            tc.tile_pool(name="psum", bufs=1, space="PSUM") as psum,
        ):
            # --- constants -------------------------------------------------
            # Causal triangle (only the 128-wide diagonal block ever needs
            # masking): cmask[p, c] = 0 if c >= p else -1e30.
            cmask = constp.tile([128, 128], F32)
            nc.gpsimd.memset(cmask, 0.0)
            nc.gpsimd.affine_select(
                out=cmask,
                in_=cmask,
                compare_op=mybir.AluOpType.is_ge,
                fill=-1.0e30,
                base=0,
                pattern=[[1, 128]],
                channel_multiplier=-1,
            )
            ident = constp.tile([128, 128], BF16)
            from concourse.masks import make_identity

            make_identity(nc, ident)

            # HAM warm-up: the PE clock-gate defaults to half rate and takes
            # ~3.4us of sustained matmul activity to open. The first real
            # matmul waits a few us for the kvs/xt DMAs, so burn that window
            # on throwaway matmuls to enter the loop at full clock.
            warm = psum.tile([128, 128], F32, tag="work", bufs=2, name="warm")
            for _ in range(32):
                nc.tensor.matmul(warm, lhsT=ident, rhs=ident, start=True, stop=True)

            # --- resident weights / tables --------------------------------
            from concourse.tile import add_dep_helper

            # Startup DMA choreography: everything issued at t=0 shares HBM
            # bandwidth, so chain the DMAs pairwise in exactly the order the
            # PE consumes them: [kvs dg | x0 dg] pairs feed the K projection
            # chunk by chunk, then Q weights, V weights, out weights.
            kvs = constp.tile([128, 2, NDC, H], BF16)
            xts_t = [None] * NT
            xts_t[0] = stream.tile([128, NDC, TQ], BF16, tag="xts", name="xts")
            pair_last = []  # last DMA of the previous chained group
            d_crit = []
            d_x0 = []
            for dg in range(4):
                da = nc.sync.dma_start(
                    out=kvs[:, 0, 4 * dg : 4 * (dg + 1)],
                    in_=kvw[:, 0, 4 * dg : 4 * (dg + 1)],
                )
                db = nc.sync.dma_start(
                    out=xts_t[0][:, 4 * dg : 4 * (dg + 1), :],
                    in_=xt[0, :, 4 * dg : 4 * (dg + 1), :],
                )
                if dg >= 2:
                    # keep two groups in flight: group dg waits on dg-2
                    for dd in (da, db):
                        add_dep_helper(
                            dd.ins, d_x0[dg - 2].ins, reason="startup chain"
                        )
                d_crit.append(da)
                d_x0.append(db)
            css = [None] * NT
            sns = [None] * NT

            def cs_dma(j, deps=()):
                css[j] = cstream.tile([128, TQ], F32, tag="cs", name="cs")
                sns[j] = cstream.tile([128, TQ], F32, tag="sn", name="sn")
                jsl = slice(j * TQ, (j + 1) * TQ)
                d1 = nc.sync.dma_start(out=css[j], in_=cost[:, jsl])
                d2 = nc.sync.dma_start(out=sns[j], in_=sint[:, jsl])
                for dep in deps:
                    add_dep_helper(d1.ins, dep.ins, reason="defer cos/sin")
                    add_dep_helper(d2.ins, dep.ins, reason="defer cos/sin")

            cs_dma(0, deps=(d_x0[1],))
            qws = constp.tile([128, 2, NDC, H], BF16)
            d_qw = []
            for n in (0, 1):
                for hg in (0, 1):
                    dd = nc.sync.dma_start(
                        out=qws[:, n, 8 * hg : 8 * (hg + 1)],
                        in_=qw[:, n, 8 * hg : 8 * (hg + 1)],
                    )
                    # chain pairwise behind the K-proj stream
                    idx = 2 * n + hg
                    prev = d_x0[1 + idx] if idx < 3 else d_qw[0]
                    add_dep_helper(dd.ins, prev.ins, reason="startup chain")
                    d_qw.append(dd)
            d_kv1 = []
            for hg in (0, 1):
                dd = nc.sync.dma_start(
                    out=kvs[:, 1, 8 * hg : 8 * (hg + 1)],
                    in_=kvw[:, 1, 8 * hg : 8 * (hg + 1)],
                )
                add_dep_helper(dd.ins, d_qw[2 + hg].ins, reason="startup chain")
                d_kv1.append(dd)
            ows = constp.tile([128, 2, 2, D], BF16)
            for hg in (0, 1):
                dd = nc.sync.dma_start(out=ows[:, hg], in_=outw[:, hg])
                add_dep_helper(dd.ins, d_kv1[hg].ins, reason="startup chain")

            # K^T halves [h-half, t] and V chunks [s-in-chunk, h | ones],
            # grown per tile. The 257th column of each V chunk is constant 1.0
            # so the PV matmul accumulates the softmax denominator for free.
            VN = H + 1  # 257
            kts = persist.tile([128, 2, T], BF16)
            vs = persist.tile([128, T // 128, VN], BF16)
            nc.vector.memset(vs[:, :, H : H + 1], 1.0)

            for i in range(NT):
                tsl = slice(i * TQ, (i + 1) * TQ)
                cos_sl = css[i]
                sin_sl = sns[i]
                if i + 1 < NT:
                    cs_dma(i + 1)
                if i > 0:
                    # x^T slice [128, 16, 512] in 4 DMAs so the first
                    # projection matmuls can start on a quarter of the data
                    xts_t[i] = stream.tile(
                        [128, NDC, TQ], BF16, tag="xts", name="xts"
                    )
                    for dg in range(4):
                        nc.sync.dma_start(
                            out=xts_t[i][:, 4 * dg : 4 * (dg + 1), :],
                            in_=xt[i, :, 4 * dg : 4 * (dg + 1), :],
                        )
                xts = xts_t[i]

                # ---- K^T projection + RoPE -------------------------------
                ctx_proj = nc.named_scope(f"t{i}_proj"); ctx_proj.__enter__()
                kp0 = psum.tile([128, TQ], F32, tag="projqk", bufs=2)
                kp1 = psum.tile([128, TQ], F32, tag="projqk", bufs=2)
                for hh, kp in ((0, kp0), (1, kp1)):
                    for d in range(NDC):
                        nc.tensor.matmul(
                            kp,
                            lhsT=kvs[:, 0, d, hh * 128 : (hh + 1) * 128],
                            rhs=xts[:, d, :],
                            start=(d == 0),
                            stop=(d == NDC - 1),
                        )
                        if i == 0 and hh == 0 and d % 4 == 3:
                            # tile 0 is DMA-paced: dep-free filler matmuls
                            # keep the PE activity monitor hot through the
                            # stalls so the clock gate opens once, early,
                            # instead of flapping back to half rate.
                            for _ in range(6):
                                nc.tensor.matmul(
                                    warm, lhsT=ident, rhs=ident,
                                    start=True, stop=True,
                                )
                _rope_pair(
                    nc, tmps, kp0, kp1, cos_sl, sin_sl,
                    kts[:, 0, tsl], kts[:, 1, tsl],
                )

                # ---- Q^T projections + RoPE (2 heads) --------------------
                qt = []
                for n in (0, 1):
                    qp0 = psum.tile([128, TQ], F32, tag="projqk", bufs=2)
                    qp1 = psum.tile([128, TQ], F32, tag="projqk", bufs=2)
                    for hh, qp in ((0, qp0), (1, qp1)):
                        for d in range(NDC):
                            nc.tensor.matmul(
                                qp,
                                lhsT=qws[:, n, d, hh * 128 : (hh + 1) * 128],
                                rhs=xts[:, d, :],
                                start=(d == 0),
                                stop=(d == NDC - 1),
                            )
                    qtn = qtp.tile([128, 2, TQ], BF16, tag="qt")
                    _rope_pair(
                        nc, tmps, qp0, qp1, cos_sl, sin_sl,
                        qtn[:, 0, :], qtn[:, 1, :],
                    )
                    qt.append(qtn)

                # ---- V projection ----------------------------------------
                for ts in range(4):
                    vp = psum.tile([128, H], F32, tag="projqk", bufs=2)
                    for d in range(NDC):
                        nc.tensor.matmul(
                            vp,
                            lhsT=xts[:, d, ts * 128 : (ts + 1) * 128],
                            rhs=kvs[:, 1, d, :],
                            start=(d == 0),
                            stop=(d == NDC - 1),
                        )
                    nc.vector.tensor_copy(vs[:, 4 * i + ts, 0:H], vp)

                ctx_proj.__exit__(None, None, None)

                # ---- attention, software-pipelined -----------------------
                # O[t-sub, h|denom] accumulates per 128-row query sub-block in
                # PSUM over s-chunks: lhsT = P^T[s, t-sub], rhs = [V | 1].
                # The flat (head, chunk) step sequence runs QK(step j+1)
                # before PV(step j); normalization of sub-block ts is emitted
                # two steps after the chunk that finalizes it, and the output
                # projection for ts streams out right after head 1's ts.
                ctx_attn = nc.named_scope(f"t{i}_attn"); ctx_attn.__enter__()
                nchunks = 4 * i + 4
                ots = [
                    otp.tile([128, 2, TQ], BF16, tag=f"ot{n}", name=f"ot{n}")
                    for n in (0, 1)
                ]
                o_ps = {}
                norm_fifo = deque()
                state = {"pv": 0}

                def out_proj_ts(ts):
                    """Output projection for rows [i*TQ+ts*128, +128)."""
                    ys = ysp.tile([128, D], BF16, tag="ys", name="ys")
                    last = i == NT - 1 and ts == 3
                    for dc in range(4):
                        py = psum.tile(
                            [128, 512], F32, tag=f"o{ts}", name="py"
                        )
                        mm = 0
                        for n in (0, 1):
                            for hh in (0, 1):
                                nc.tensor.matmul(
                                    py,
                                    lhsT=ots[n][:, hh, ts * 128 : (ts + 1) * 128],
                                    rhs=ows[:, n, hh, dc * 512 : (dc + 1) * 512],
                                    start=(mm == 0),
                                    stop=(mm == 3),
                                )
                                mm += 1
                        if last:
                            # kernel tail: split the copy between DVE and ACT
                            # and DMA per 512-col chunk to shorten the drain
                            eng = nc.vector if dc % 2 == 0 else nc.scalar
                            if dc % 2 == 0:
                                eng.tensor_copy(
                                    ys[:, dc * 512 : (dc + 1) * 512], py
                                )
                            else:
                                eng.copy(ys[:, dc * 512 : (dc + 1) * 512], py)
                            nc.sync.dma_start(
                                out=y[
                                    i * TQ + ts * 128 : i * TQ + (ts + 1) * 128,
                                    dc * 512 : (dc + 1) * 512,
                                ],
                                in_=ys[:, dc * 512 : (dc + 1) * 512],
                            )
                        else:
                            nc.vector.tensor_copy(
                                ys[:, dc * 512 : (dc + 1) * 512], py
                            )
                    if not last:
                        nc.sync.dma_start(
                            out=y[i * TQ + ts * 128 : i * TQ + (ts + 1) * 128, :],
                            in_=ys,
                        )

                def emit_norm(n, ts):
                    """Normalize o_ps[n][ts] by its denominator column and
                    transpose into ots[n]; stream the output projection once
                    head 1's slice lands."""
                    tail = i == NT - 1 and ts >= 2
                    rd = tmps.tile([128, 1], F32, tag="rd", bufs=4)
                    nc.vector.reciprocal(rd, o_ps[n][ts][:, H : H + 1])
                    ob = tmps.tile([128, H], BF16, tag="ob", bufs=3)
                    if tail:
                        # kernel tail: ACT is idle, DVE is the critical chain
                        nc.scalar.mul(ob, o_ps[n][ts][:, 0:H], rd)
                    else:
                        nc.vector.tensor_scalar_mul(ob, o_ps[n][ts][:, 0:H], rd)
                    for hh in (0, 1):
                        tp = psum.tile(
                            [128, 128], BF16, tag="projqk", bufs=2, name="tp"
                        )
                        nc.tensor.transpose(
                            tp, ob[:, 128 * hh : 128 * (hh + 1)], ident
                        )
                        if tail:
                            nc.scalar.copy(
                                ots[n][:, hh, 128 * ts : 128 * (ts + 1)], tp
                            )
                        else:
                            nc.vector.tensor_copy(
                                ots[n][:, hh, 128 * ts : 128 * (ts + 1)], tp
                            )
                    if n == 1:
                        out_proj_ts(ts)

                def emit_pv(n, k, pt):
                    """PV matmuls for chunk k of head n, then any normalize
                    whose finalizing chunk's PV was emitted a step ago."""
                    if n not in o_ps:
                        o_ps[n] = [
                            psum.tile([128, VN], F32, tag=f"o{ts}", name=f"o{ts}")
                            for ts in range(4)
                        ]
                    q_ = max(0, k - 4 * i)
                    for ts in range(q_, 4):
                        nc.tensor.matmul(
                            o_ps[n][ts],
                            lhsT=pt[:, 128 * ts : 128 * (ts + 1)],
                            rhs=vs[:, k, :],
                            start=(k == 0),
                            stop=(k == 4 * i + ts),
                        )
                    state["pv"] += 1
                    if k >= 4 * i:
                        norm_fifo.append((n, k - 4 * i, state["pv"]))
                    while norm_fifo and norm_fifo[0][2] <= state["pv"] - 1:
                        nn, ts, _ = norm_fifo.popleft()
                        emit_norm(nn, ts)

                prev = None
                for n in (0, 1):
                    for k in range(nchunks):
                        q_ = max(0, k - 4 * i)
                        col0 = 128 * q_
                        ksl = slice(k * 128, (k + 1) * 128)
                        pl = psum.tile([128, TQ], F32, tag="work", bufs=2, name="pl")
                        nc.tensor.matmul(
                            pl[:, col0:],
                            lhsT=kts[:, 0, ksl],
                            rhs=qt[n][:, 0, col0:],
                            start=True,
                            stop=False,
                        )
                        nc.tensor.matmul(
                            pl[:, col0:],
                            lhsT=kts[:, 1, ksl],
                            rhs=qt[n][:, 1, col0:],
                            start=False,
                            stop=True,
                        )
                        pt = ptp.tile([128, TQ], BF16, tag="pt", bufs=4, name="pt")
                        nc.scalar.activation(pt[:, col0:], pl[:, col0:], EXP)
                        if k >= 4 * i:
                            # only the 128-wide diagonal block needs masking;
                            # exp(masked logit) == 0, so zero the upper
                            # triangle of P^T post-exp on the idle GpSimd
                            # engine (keeps DVE out of the QK->exp->PV chain)
                            nc.gpsimd.affine_select(
                                out=pt[:, col0 : col0 + 128],
                                in_=pt[:, col0 : col0 + 128],
                                compare_op=mybir.AluOpType.is_ge,
                                fill=0.0,
                                base=0,
                                pattern=[[1, 128]],
                                channel_multiplier=-1,
                            )
                        if prev is not None:
                            emit_pv(*prev)
                        prev = (n, k, pt)
                emit_pv(*prev)
                while norm_fifo:
                    nn, ts, _ = norm_fifo.popleft()
                    emit_norm(nn, ts)
                ctx_attn.__exit__(None, None, None)
    n = _split_excess_waits(nc)
    print(f"kernel build: split {n} excess waits")
    return nc


def _is_causal(mask):
    """mask: [B, T, T] bool — check it's exactly the causal tril mask."""
    tri = np.tril(np.ones((T, T), dtype=bool))
    return all(np.array_equal(mask[b], tri) for b in range(mask.shape[0]))


def _numpy_reference(x, segment_pos, attn_mask, q_w, kv_w, out_w):
    """Slow exact fallback for non-causal masks (matches reference.py)."""
    x = np.asarray(x, np.float32)
    out = np.zeros((B, T, D), np.float32)
    j = np.arange(HH, dtype=np.float32)
    timescale = 10000.0 ** (2.0 * j / H)
    for b in range(B):
        ang = segment_pos[b][:, None].astype(np.float32) / timescale[None, :]
        cos, sin = np.cos(ang), np.sin(ang)  # [T, 128]
        k = x[b] @ kv_w[0, 0]  # [T, H]
        v = x[b] @ kv_w[1, 0]
        k = np.concatenate(
            [k[:, :HH] * cos - k[:, HH:] * sin, k[:, HH:] * cos + k[:, :HH] * sin], 1
        )
        for n in range(N):
            q = x[b] @ q_w[n]
            q = np.concatenate(
                [q[:, :HH] * cos - q[:, HH:] * sin, q[:, HH:] * cos + q[:, :HH] * sin],
                1,
            ) * (H ** -0.5)
            logits = q @ k.T  # [T, T]
            logits = np.where(attn_mask[b], logits, -2.3819763e38)
            logits -= logits.max(-1, keepdims=True)
            p = np.exp(logits)
            p /= p.sum(-1, keepdims=True)
            out[b] += (p.astype(np.float32) @ v) @ out_w[n]
    return out


def kernel(x, segment_pos, attn_mask, q_w, kv_w, out_w):
    global LAST_RESULT
    x = np.asarray(x)
    segment_pos = np.asarray(segment_pos)
    attn_mask = np.asarray(attn_mask)
    q_w = np.asarray(q_w)
    kv_w = np.asarray(kv_w)
    out_w = np.asarray(out_w)
    assert x.shape == (B, T, D) and q_w.shape == (N, D, H)

    if not _is_causal(attn_mask):
        return _numpy_reference(x, segment_pos, attn_mask, q_w, kv_w, out_w)

    from concourse.bass_utils import run_bass_kernel_spmd

    if "nc" not in _CACHE:
        _CACHE["nc"] = _build_nc()
    nc = _CACHE["nc"]

    bf16 = ml_dtypes.bfloat16

    def dxh_pre(w):  # [2, D, H] -> [128, 2, NDC, H] (partition-major)
        return np.ascontiguousarray(
            w.reshape(2, NDC, 128, H).transpose(2, 0, 1, 3)
        ).astype(bf16)

    # Per-batch host prep
    xts, coss, sins = [], [], []
    j = np.arange(HH, dtype=np.float32)
    timescale = 10000.0 ** (2.0 * j / H)
    for b in range(B):
        # x[b] [T, D] -> x^T tiles [NT, 128(dp), NDC, TQ]
        xtp = np.ascontiguousarray(
            x[b].T.reshape(NDC, 128, NT, TQ).transpose(2, 1, 0, 3)
        ).astype(bf16)
        xts.append(xtp)
        ang = segment_pos[b][None, :].astype(np.float32) / timescale[:, None]
        coss.append(np.cos(ang).astype(np.float32))
        sins.append(np.sin(ang).astype(np.float32))
    kvw_host = dxh_pre(kv_w[:, 0])
    qw_scaled = q_w * np.float32(H ** -0.5)  # [N, D, H]
    # out_w [n, H, D] -> [128(hp), 2(n), 2(hh), D]
    outw_all = [
        np.ascontiguousarray(
            out_w[2 * m : 2 * m + 2].reshape(2, 2, 128, D).transpose(2, 0, 1, 3)
        ).astype(bf16)
        for m in range(4)
    ]
    qw_all = [dxh_pre(qw_scaled[2 * m : 2 * m + 2]) for m in range(4)]

    in_maps = []
    for c in range(NCORES):
        b, m = c // 4, c % 4
        in_maps.append(
            {
                "xt": xts[b],
                "qw": qw_all[m],
                "kvw": kvw_host,
                "outw": outw_all[m],
                "cost": coss[b],
                "sint": sins[b],
            }
        )

    trace = bool(int(os.environ.get("KERNEL_TRACE", "0")))
    res = run_bass_kernel_spmd(nc, in_maps, core_ids=list(range(NCORES)), trace=trace)
    LAST_RESULT = res

    out = np.zeros((B, T, D), np.float32)
    for c in range(NCORES):
        out[c // 4] += res.results[c]["y"].astype(np.float32)
    return out


# revision 26
# speedup vs baseline: 1.0636x; 1.0059x over previous
"""Trainium2 Bass kernel for GQA attention (B=2, T=4096, D=2048, N=8 q-heads,
K=1 kv-head, H=256) with RoPE + causal mask + output projection.

Sharding: data-parallel on batch (2) x tensor-parallel on query heads
(4 groups of 2 heads) = 8 cores. Each core computes a partial output
y_c = sum_{n in its 2 heads} softmax(q_n k^T) v @ out_w[n] for its batch;
the host sums the 4 partials per batch. (A cross-core AllReduce KV-dedup
was tried and reverted: collective SDMA traffic trips a GPIO power
throttle that caps the PE clock at 13/16 for most of the run, costing
more than the deduplicated projection work saved.)

The device kernel is identical on every core (single NEFF, SPMD); per-core
behaviour comes only from per-core input data:
  xt   [2048, 4096] bf16 : x[b]^T  (pre-transposed + bf16 on host)
  qw   [2, 2048, 256] bf16 : q_w for the core's 2 heads, pre-scaled by H^-0.5
  kvw  [2, 2048, 256] bf16 : k/v projection weights (shared kv head)
  outw [2, 256, 2048] bf16 : out_w for the core's 2 heads
  cost/sint [128, 4096] f32 : RoPE cos/sin tables (timescale j x position t)
Output: y [4096, 2048] bf16 partial (summed in f32 on host).

Flash-attention layout: everything transposed (S^T = K^T^T-contraction) so
softmax statistics land in matmuls:
  K^T,Q^T [h, t] from projections directly; logits S^T [s-chunk 128, t 512]
  in PSUM; exp on ACT -> P^T bf16; PV as pt-stationary matmul giving
  O [t-sub, h | denom] accumulated over s-chunks in PSUM; denominator via
  a constant-1 column appended to V; normalization by per-partition DVE
  scale, then PE transpose to O^T for the output projection.

Scheduling: the (head, chunk) loop is software-pipelined one step deep --
QK(k+1) is issued on the PE before PV(k) -- so the QK->exp->PV chain
latency (ACT engine) is hidden behind the next chunk's QK matmuls.
Normalization of query sub-block ts is issued 2 steps after the chunk that
finalizes its PSUM row, and the output projection streams out per 128-row
sub-block as soon as both heads' normalized O^T slices exist.
"""

import os
from collections import deque

import numpy as np
import ml_dtypes

B, T, D, N, H = 2, 4096, 2048, 8, 256
NCORES = 8
HH = H // 2  # 128, also the RoPE pair offset and partition size
TQ = 512     # query-tile columns (moving dim of logits matmul)
NT = T // TQ # 8 query tiles
NDC = D // 128  # 16 contraction chunks over D

_CACHE = {}
LAST_RESULT = None  # BassKernelResults of the most recent device run (for test harness)


def _split_excess_waits(nc):
    """The walrus in this container accepts at most 1 sync-wait per
    instruction (2 for EventSemaphore); Tile attaches one wait per producer
    semaphore. Hoist excess waits onto injected same-engine NOPs immediately
    before the instruction (engine queues are in-order, so waiting A then B
    sequentially == waiting {A,B} at once)."""
    import bass_rust
    import concourse.mybir as mybir

    n_split = 0
    for f in nc.m.functions:
        for bb in f.blocks:
            insts = bb.instructions
            out = []
            changed = False
            for inst in insts:
                si = inst.sync_info
                waits = list(si.on_wait) if si is not None and si.on_wait else []
                cap = 2 if isinstance(inst, mybir.InstEventSemaphore) else 1
                if len(waits) > cap:
                    changed = True
                    for w in waits[:-cap]:
                        nop = mybir.InstNoOp(
                            name=f"waitsplit_{n_split}", ins=[], outs=[]
                        )
                        n_split += 1
                        nop.engine = inst.engine
                        nop.sync_info = bass_rust.SyncInfo(on_wait=[w], on_update=[])
                        out.append(nop)
                    inst.sync_info = bass_rust.SyncInfo(
                        on_wait=waits[-cap:], on_update=si.on_update
                    )
                out.append(inst)
            if changed:
                insts[:] = out
                if bb.instructions[0].name != out[0].name or len(bb.instructions) != len(out):
                    raise RuntimeError("basic block instruction list not live-mutable")
    return n_split


def _rope_pair(nc, tmps, p0, p1, cos_s, sin_s, out0, out1):
    """out0 = p0*cos - p1*sin ; out1 = p1*cos + p0*sin  (RoPE half-pair).
    p0/p1: [128, L] f32 PSUM; copied to SBUF first (frees the PSUM bank
    after ~1 ACT op instead of after 4 DVE ops). cos/sin: [128, L] f32
    SBUF, out0/out1: [128, L] bf16 SBUF."""
    import concourse.mybir as mybir

    L = p0.shape[-1]
    c0 = tmps.tile([128, TQ], mybir.dt.float32, tag="projc")
    c1 = tmps.tile([128, TQ], mybir.dt.float32, tag="projc")
    nc.scalar.copy(c0[:, :L], p0)
    nc.vector.tensor_copy(c1[:, :L], p1)
    t0 = tmps.tile([128, TQ], mybir.dt.float32, tag="ropetmp")
    t1 = tmps.tile([128, TQ], mybir.dt.float32, tag="ropetmp")
    nc.vector.tensor_mul(t0[:, :L], c0[:, :L], cos_s)
    nc.vector.tensor_mul(t1[:, :L], c1[:, :L], sin_s)
    nc.vector.tensor_sub(out0, t0[:, :L], t1[:, :L])
    t2 = tmps.tile([128, TQ], mybir.dt.float32, tag="ropetmp")
    t3 = tmps.tile([128, TQ], mybir.dt.float32, tag="ropetmp")
    nc.vector.tensor_mul(t2[:, :L], c1[:, :L], cos_s)
    nc.vector.tensor_mul(t3[:, :L], c0[:, :L], sin_s)
    nc.vector.tensor_add(out1, t2[:, :L], t3[:, :L])


def _build_nc():
    import concourse.bass as bass
    import concourse.mybir as mybir
    import concourse.tile as tile

    F32 = mybir.dt.float32
    BF16 = mybir.dt.bfloat16
    EXP = mybir.ActivationFunctionType.Exp

    nc = bass.Bass()
    # All inputs are pre-arranged on the host to the exact SBUF layout so
    # every DMA moves >=2KB-contiguous per-partition lines at full rate.
    xt = nc.dram_tensor("xt", [NT, 128, NDC, TQ], BF16, kind="ExternalInput")
    qw = nc.dram_tensor("qw", [128, 2, NDC, H], BF16, kind="ExternalInput")
    kvw = nc.dram_tensor("kvw", [128, 2, NDC, H], BF16, kind="ExternalInput")
    outw = nc.dram_tensor("outw", [128, 2, 2, D], BF16, kind="ExternalInput")
    cost = nc.dram_tensor("cost", [HH, T], F32, kind="ExternalInput")
    sint = nc.dram_tensor("sint", [HH, T], F32, kind="ExternalInput")
    y = nc.dram_tensor("y", [T, D], BF16, kind="ExternalOutput")

    with tile.TileContext(nc) as tc:
        with (
            tc.tile_pool(name="const", bufs=1) as constp,
            tc.tile_pool(name="persist", bufs=1) as persist,
            tc.tile_pool(name="stream", bufs=2) as stream,
            tc.tile_pool(name="cstream", bufs=2) as cstream,
            tc.tile_pool(name="qtp", bufs=4) as qtp,
            tc.tile_pool(name="tmps", bufs=3) as tmps,
            tc.tile_pool(name="ptp", bufs=3) as ptp,
            tc.tile_pool(name="otp", bufs=2) as otp,
            tc.tile_pool(name="ysp", bufs=2) as ysp,
            tc.tile_pool(name="psum", bufs=1, space="PSUM") as psum,
        ):
            # --- constants -------------------------------------------------
            # Causal triangle (only the 128-wide diagonal block ever needs
            # masking): cmask[p, c] = 0 if c >= p else -1e30.
            cmask = constp.tile([128, 128], F32)
            nc.gpsimd.memset(cmask, 0.0)
            nc.gpsimd.affine_select(
                out=cmask,
                in_=cmask,
                compare_op=mybir.AluOpType.is_ge,
                fill=-1.0e30,
                base=0,
                pattern=[[1, 128]],
                channel_multiplier=-1,
            )
            # HAM warm-up: the PE clock-gate defaults to half rate and takes
            # ~3.4us of sustained matmul activity to open. The first real
            # matmul waits a few us for the kvs/xt DMAs, so burn that window
            # on throwaway matmuls to enter the loop at full clock. The junk
            # operand only needs a memset (by gpsimd, free at t=0), so the
            # PE starts ~2.5us before make_identity's chain would allow.
            junk = constp.tile([128, 128], BF16)
            nc.gpsimd.memset(junk, 0.0)
            warm = psum.tile([128, 128], F32, tag="work", bufs=2, name="warm")
            for _ in range(32):
                nc.tensor.matmul(warm, lhsT=junk, rhs=junk, start=True, stop=True)

            ident = constp.tile([128, 128], BF16)
            from concourse.masks import make_identity

            make_identity(nc, ident)

            # --- resident weights / tables --------------------------------
            from concourse.tile import add_dep_helper

            # Startup DMA choreography: everything issued at t=0 shares HBM
            # bandwidth, so chain the DMAs pairwise in exactly the order the
            # PE consumes them: [kvs dg | x0 dg] pairs feed the K projection
            # chunk by chunk, then Q weights, V weights, out weights.
            kvs = constp.tile([128, 2, NDC, H], BF16)
            xts_t = [None] * NT
            xts_t[0] = stream.tile([128, NDC, TQ], BF16, tag="xts", name="xts")
            pair_last = []  # last DMA of the previous chained group
            d_crit = []
            d_x0 = []
            for dg in range(4):
                da = nc.sync.dma_start(
                    out=kvs[:, 0, 4 * dg : 4 * (dg + 1)],
                    in_=kvw[:, 0, 4 * dg : 4 * (dg + 1)],
                )
                db = nc.sync.dma_start(
                    out=xts_t[0][:, 4 * dg : 4 * (dg + 1), :],
                    in_=xt[0, :, 4 * dg : 4 * (dg + 1), :],
                )
                if dg >= 2:
                    # keep two groups in flight: group dg waits on dg-2
                    for dd in (da, db):
                        add_dep_helper(
                            dd.ins, d_x0[dg - 2].ins, reason="startup chain"
                        )
                d_crit.append(da)
                d_x0.append(db)
            css = [None] * NT
            sns = [None] * NT

            def cs_dma(j, deps=()):
                css[j] = cstream.tile([128, TQ], F32, tag="cs", name="cs")
                sns[j] = cstream.tile([128, TQ], F32, tag="sn", name="sn")
                jsl = slice(j * TQ, (j + 1) * TQ)
                d1 = nc.sync.dma_start(out=css[j], in_=cost[:, jsl])
                d2 = nc.sync.dma_start(out=sns[j], in_=sint[:, jsl])
                for dep in deps:
                    add_dep_helper(d1.ins, dep.ins, reason="defer cos/sin")
                    add_dep_helper(d2.ins, dep.ins, reason="defer cos/sin")

            cs_dma(0, deps=(d_x0[1],))
            qws = constp.tile([128, 2, NDC, H], BF16)
            d_qw = []
            for n in (0, 1):
                for hg in (0, 1):
                    dd = nc.sync.dma_start(
                        out=qws[:, n, 8 * hg : 8 * (hg + 1)],
                        in_=qw[:, n, 8 * hg : 8 * (hg + 1)],
                    )
                    # chain pairwise behind the K-proj stream
                    idx = 2 * n + hg
                    prev = d_x0[1 + idx] if idx < 3 else d_qw[0]
                    add_dep_helper(dd.ins, prev.ins, reason="startup chain")
                    d_qw.append(dd)
            d_kv1 = []
            for hg in (0, 1):
                dd = nc.sync.dma_start(
                    out=kvs[:, 1, 8 * hg : 8 * (hg + 1)],
                    in_=kvw[:, 1, 8 * hg : 8 * (hg + 1)],
                )
                add_dep_helper(dd.ins, d_qw[2 + hg].ins, reason="startup chain")
                d_kv1.append(dd)
            ows = constp.tile([128, 2, 2, D], BF16)
            for hg in (0, 1):
                dd = nc.sync.dma_start(out=ows[:, hg], in_=outw[:, hg])
                add_dep_helper(dd.ins, d_kv1[hg].ins, reason="startup chain")

            # K^T halves [h-half, t] and V chunks [s-in-chunk, h | ones],
            # grown per tile. The 257th column of each V chunk is constant 1.0
            # so the PV matmul accumulates the softmax denominator for free.
            VN = H + 1  # 257
            kts = persist.tile([128, 2, T], BF16)
            vs = persist.tile([128, T // 128, VN], BF16)
            nc.vector.memset(vs[:, :, H : H + 1], 1.0)

            for i in range(NT):
                tsl = slice(i * TQ, (i + 1) * TQ)
                cos_sl = css[i]
                sin_sl = sns[i]
                if i + 1 < NT:
                    cs_dma(i + 1)
                if i > 0:
                    # x^T slice [128, 16, 512] in 4 DMAs so the first
                    # projection matmuls can start on a quarter of the data
                    xts_t[i] = stream.tile(
                        [128, NDC, TQ], BF16, tag="xts", name="xts"
                    )
                    for dg in range(4):
                        nc.sync.dma_start(
                            out=xts_t[i][:, 4 * dg : 4 * (dg + 1), :],
                            in_=xt[i, :, 4 * dg : 4 * (dg + 1), :],
                        )
                xts = xts_t[i]

                # ---- K^T projection + RoPE -------------------------------
                ctx_proj = nc.named_scope(f"t{i}_proj"); ctx_proj.__enter__()
                kp0 = psum.tile([128, TQ], F32, tag="projqk", bufs=2)
                kp1 = psum.tile([128, TQ], F32, tag="projqk", bufs=2)
                for hh, kp in ((0, kp0), (1, kp1)):
                    for d in range(NDC):
                        nc.tensor.matmul(
                            kp,
                            lhsT=kvs[:, 0, d, hh * 128 : (hh + 1) * 128],
                            rhs=xts[:, d, :],
                            start=(d == 0),
                            stop=(d == NDC - 1),
                        )
                        if i == 0 and hh == 0 and d % 4 == 3:
                            # tile 0 is DMA-paced: dep-free filler matmuls
                            # keep the PE activity monitor hot through the
                            # stalls so the clock gate opens once, early,
                            # instead of flapping back to half rate.
                            for _ in range(6):
                                nc.tensor.matmul(
                                    warm, lhsT=ident, rhs=ident,
                                    start=True, stop=True,
                                )
                _rope_pair(
                    nc, tmps, kp0, kp1, cos_sl, sin_sl,
                    kts[:, 0, tsl], kts[:, 1, tsl],
                )

                # ---- Q^T projections + RoPE (2 heads) --------------------
                qt = []
                for n in (0, 1):
                    qp0 = psum.tile([128, TQ], F32, tag="projqk", bufs=2)
                    qp1 = psum.tile([128, TQ], F32, tag="projqk", bufs=2)
                    for hh, qp in ((0, qp0), (1, qp1)):
                        for d in range(NDC):
                            nc.tensor.matmul(
                                qp,
                                lhsT=qws[:, n, d, hh * 128 : (hh + 1) * 128],
                                rhs=xts[:, d, :],
                                start=(d == 0),
                                stop=(d == NDC - 1),
                            )
                    qtn = qtp.tile([128, 2, TQ], BF16, tag="qt")
                    _rope_pair(
                        nc, tmps, qp0, qp1, cos_sl, sin_sl,
                        qtn[:, 0, :], qtn[:, 1, :],
                    )
                    qt.append(qtn)

                # ---- V projection ----------------------------------------
                for ts in range(4):
                    vp = psum.tile([128, H], F32, tag="projqk", bufs=2)
                    for d in range(NDC):
                        nc.tensor.matmul(
                            vp,
                            lhsT=xts[:, d, ts * 128 : (ts + 1) * 128],
                            rhs=kvs[:, 1, d, :],
                            start=(d == 0),
                            stop=(d == NDC - 1),
                        )
                    nc.vector.tensor_copy(vs[:, 4 * i + ts, 0:H], vp)

                ctx_proj.__exit__(None, None, None)

                # ---- attention, software-pipelined -----------------------
                # O[t-sub, h|denom] accumulates per 128-row query sub-block in
                # PSUM over s-chunks: lhsT = P^T[s, t-sub], rhs = [V | 1].
                # The flat (head, chunk) step sequence runs QK(step j+1)
                # before PV(step j); normalization of sub-block ts is emitted
                # two steps after the chunk that finalizes it, and the output
                # projection for ts streams out right after head 1's ts.
                ctx_attn = nc.named_scope(f"t{i}_attn"); ctx_attn.__enter__()
                nchunks = 4 * i + 4
                ots = [
                    otp.tile([128, 2, TQ], BF16, tag=f"ot{n}", name=f"ot{n}")
                    for n in (0, 1)
                ]
                o_ps = {}
                norm_fifo = deque()
                state = {"pv": 0}

                def out_proj_ts(ts):
                    """Output projection for rows [i*TQ+ts*128, +128)."""
                    ys = ysp.tile([128, D], BF16, tag="ys", name="ys")
                    last = i == NT - 1 and ts == 3
                    for dc in range(4):
                        py = psum.tile(
                            [128, 512], F32, tag=f"o{ts}", name="py"
                        )
                        mm = 0
                        for n in (0, 1):
                            for hh in (0, 1):
                                nc.tensor.matmul(
                                    py,
                                    lhsT=ots[n][:, hh, ts * 128 : (ts + 1) * 128],
                                    rhs=ows[:, n, hh, dc * 512 : (dc + 1) * 512],
                                    start=(mm == 0),
                                    stop=(mm == 3),
                                )
                                mm += 1
                        if last:
                            # kernel tail: split the copy between DVE and ACT
                            # and DMA per 512-col chunk to shorten the drain
                            eng = nc.vector if dc % 2 == 0 else nc.scalar
                            if dc % 2 == 0:
                                eng.tensor_copy(
                                    ys[:, dc * 512 : (dc + 1) * 512], py
                                )
                            else:
                                eng.copy(ys[:, dc * 512 : (dc + 1) * 512], py)
                            nc.sync.dma_start(
                                out=y[
                                    i * TQ + ts * 128 : i * TQ + (ts + 1) * 128,
                                    dc * 512 : (dc + 1) * 512,
                                ],
                                in_=ys[:, dc * 512 : (dc + 1) * 512],
                            )
                        else:
                            nc.vector.tensor_copy(
                                ys[:, dc * 512 : (dc + 1) * 512], py
                            )
                    if not last:
                        nc.sync.dma_start(
                            out=y[i * TQ + ts * 128 : i * TQ + (ts + 1) * 128, :],
                            in_=ys,
                        )

                def emit_norm(n, ts):
                    """Normalize o_ps[n][ts] by its denominator column and
                    transpose into ots[n]; stream the output projection once
                    head 1's slice lands."""
                    tail = i == NT - 1 and ts >= 2
                    rd = tmps.tile([128, 1], F32, tag="rd", bufs=4)
                    nc.vector.reciprocal(rd, o_ps[n][ts][:, H : H + 1])
                    ob = tmps.tile([128, H], BF16, tag="ob", bufs=3)
                    if tail:
                        # kernel tail: ACT is idle, DVE is the critical chain
                        nc.scalar.mul(ob, o_ps[n][ts][:, 0:H], rd)
                    else:
                        nc.vector.tensor_scalar_mul(ob, o_ps[n][ts][:, 0:H], rd)
                    for hh in (0, 1):
                        tp = psum.tile(
                            [128, 128], BF16, tag="projqk", bufs=2, name="tp"
                        )
                        nc.tensor.transpose(
                            tp, ob[:, 128 * hh : 128 * (hh + 1)], ident
                        )
                        if tail:
                            nc.scalar.copy(
                                ots[n][:, hh, 128 * ts : 128 * (ts + 1)], tp
                            )
                        else:
                            nc.vector.tensor_copy(
                                ots[n][:, hh, 128 * ts : 128 * (ts + 1)], tp
                            )
                    if n == 1:
                        out_proj_ts(ts)

                def emit_pv(n, k, pt):
                    """PV matmuls for chunk k of head n, then any normalize
                    whose finalizing chunk's PV was emitted a step ago."""
                    if n not in o_ps:
                        o_ps[n] = [
                            psum.tile([128, VN], F32, tag=f"o{ts}", name=f"o{ts}")
                            for ts in range(4)
                        ]
                    q_ = max(0, k - 4 * i)
                    for ts in range(q_, 4):
                        nc.tensor.matmul(
                            o_ps[n][ts],
                            lhsT=pt[:, 128 * ts : 128 * (ts + 1)],
                            rhs=vs[:, k, :],
                            start=(k == 0),
                            stop=(k == 4 * i + ts),
                        )
                    state["pv"] += 1
                    if k >= 4 * i:
                        norm_fifo.append((n, k - 4 * i, state["pv"]))
                    while norm_fifo and norm_fifo[0][2] <= state["pv"] - 1:
                        nn, ts, _ = norm_fifo.popleft()
                        emit_norm(nn, ts)

                prev = None
                for n in (0, 1):
                    for k in range(nchunks):
                        q_ = max(0, k - 4 * i)
                        col0 = 128 * q_
                        ksl = slice(k * 128, (k + 1) * 128)
                        pl = psum.tile([128, TQ], F32, tag="work", bufs=2, name="pl")
                        nc.tensor.matmul(
                            pl[:, col0:],
                            lhsT=kts[:, 0, ksl],
                            rhs=qt[n][:, 0, col0:],
                            start=True,
                            stop=False,
                        )
                        nc.tensor.matmul(
                            pl[:, col0:],
                            lhsT=kts[:, 1, ksl],
                            rhs=qt[n][:, 1, col0:],
                            start=False,
                            stop=True,
                        )
                        pt = ptp.tile([128, TQ], BF16, tag="pt", bufs=4, name="pt")
                        nc.scalar.activation(pt[:, col0:], pl[:, col0:], EXP)
                        if k >= 4 * i:
                            # only the 128-wide diagonal block needs masking;
                            # exp(masked logit) == 0, so zero the upper
                            # triangle of P^T post-exp on the idle GpSimd
                            # engine (keeps DVE out of the QK->exp->PV chain)
                            nc.gpsimd.affine_select(
                                out=pt[:, col0 : col0 + 128],
                                in_=pt[:, col0 : col0 + 128],
                                compare_op=mybir.AluOpType.is_ge,
                                fill=0.0,
                                base=0,
                                pattern=[[1, 128]],
                                channel_multiplier=-1,
                            )
                        if prev is not None:
                            emit_pv(*prev)
                        prev = (n, k, pt)
                emit_pv(*prev)
                while norm_fifo:
                    nn, ts, _ = norm_fifo.popleft()
                    emit_norm(nn, ts)
                ctx_attn.__exit__(None, None, None)
    n = _split_excess_waits(nc)
    print(f"kernel build: split {n} excess waits")
    return nc


def _is_causal(mask):
    """mask: [B, T, T] bool — check it's exactly the causal tril mask."""
    tri = np.tril(np.ones((T, T), dtype=bool))
    return all(np.array_equal(mask[b], tri) for b in range(mask.shape[0]))


def _numpy_reference(x, segment_pos, attn_mask, q_w, kv_w, out_w):
    """Slow exact fallback for non-causal masks (matches reference.py)."""
    x = np.asarray(x, np.float32)
    out = np.zeros((B, T, D), np.float32)
    j = np.arange(HH, dtype=np.float32)
    timescale = 10000.0 ** (2.0 * j / H)
    for b in range(B):
        ang = segment_pos[b][:, None].astype(np.float32) / timescale[None, :]
        cos, sin = np.cos(ang), np.sin(ang)  # [T, 128]
        k = x[b] @ kv_w[0, 0]  # [T, H]
        v = x[b] @ kv_w[1, 0]
        k = np.concatenate(
            [k[:, :HH] * cos - k[:, HH:] * sin, k[:, HH:] * cos + k[:, :HH] * sin], 1
        )
        for n in range(N):
            q = x[b] @ q_w[n]
            q = np.concatenate(
                [q[:, :HH] * cos - q[:, HH:] * sin, q[:, HH:] * cos + q[:, :HH] * sin],
                1,
            ) * (H ** -0.5)
            logits = q @ k.T  # [T, T]
            logits = np.where(attn_mask[b], logits, -2.3819763e38)
            logits -= logits.max(-1, keepdims=True)
            p = np.exp(logits)
            p /= p.sum(-1, keepdims=True)
            out[b] += (p.astype(np.float32) @ v) @ out_w[n]
    return out


def kernel(x, segment_pos, attn_mask, q_w, kv_w, out_w):
    global LAST_RESULT
    x = np.asarray(x)
    segment_pos = np.asarray(segment_pos)
    attn_mask = np.asarray(attn_mask)
    q_w = np.asarray(q_w)
    kv_w = np.asarray(kv_w)
    out_w = np.asarray(out_w)
    assert x.shape == (B, T, D) and q_w.shape == (N, D, H)

    if not _is_causal(attn_mask):
        return _numpy_reference(x, segment_pos, attn_mask, q_w, kv_w, out_w)

    from concourse.bass_utils import run_bass_kernel_spmd

    if "nc" not in _CACHE:
        _CACHE["nc"] = _build_nc()
    nc = _CACHE["nc"]

    bf16 = ml_dtypes.bfloat16

    def dxh_pre(w):  # [2, D, H] -> [128, 2, NDC, H] (partition-major)
        return np.ascontiguousarray(
            w.reshape(2, NDC, 128, H).transpose(2, 0, 1, 3)
        ).astype(bf16)

    # Per-batch host prep
    xts, coss, sins = [], [], []
    j = np.arange(HH, dtype=np.float32)
    timescale = 10000.0 ** (2.0 * j / H)
    for b in range(B):
        # x[b] [T, D] -> x^T tiles [NT, 128(dp), NDC, TQ]
        xtp = np.ascontiguousarray(
            x[b].T.reshape(NDC, 128, NT, TQ).transpose(2, 1, 0, 3)
        ).astype(bf16)
        xts.append(xtp)
        ang = segment_pos[b][None, :].astype(np.float32) / timescale[:, None]
        coss.append(np.cos(ang).astype(np.float32))
        sins.append(np.sin(ang).astype(np.float32))
    kvw_host = dxh_pre(kv_w[:, 0])
    qw_scaled = q_w * np.float32(H ** -0.5)  # [N, D, H]
    # out_w [n, H, D] -> [128(hp), 2(n), 2(hh), D]
    outw_all = [
        np.ascontiguousarray(
            out_w[2 * m : 2 * m + 2].reshape(2, 2, 128, D).transpose(2, 0, 1, 3)
        ).astype(bf16)
        for m in range(4)
    ]
    qw_all = [dxh_pre(qw_scaled[2 * m : 2 * m + 2]) for m in range(4)]

    in_maps = []
    for c in range(NCORES):
        b, m = c // 4, c % 4
        in_maps.append(
            {
                "xt": xts[b],
                "qw": qw_all[m],
                "kvw": kvw_host,
                "outw": outw_all[m],
                "cost": coss[b],
                "sint": sins[b],
            }
        )

    trace = bool(int(os.environ.get("KERNEL_TRACE", "0")))
    res = run_bass_kernel_spmd(nc, in_maps, core_ids=list(range(NCORES)), trace=trace)
    LAST_RESULT = res

    out = np.zeros((B, T, D), np.float32)
    for c in range(NCORES):
        out[c // 4] += res.results[c]["y"].astype(np.float32)
    return out
